# revision 8
# baseline (speedup 1.0000x reference)
"""Trainium2 Bass kernel for ChebyNet (K=1) forward pass.

ChebConv with K=1 reduces to a plain linear layer on the T0 (identity) term,
so edge_index / edge_weight never enter the math. The network is:

    h1 = x @ W1.T (+b1, dropped: BN cancels additive bias) -> BN -> ReLU
    h2 = h1 @ W2.T (+b2, dropped)                          -> BN -> ReLU
    h3 = relu(h2 @ Wl1.T + bl1)
    out = log_softmax(h3 @ Wl2.T + bl2, axis=1)

Sharding: nodes (N=50000) split across 8 NeuronCores (R=6250 rows each).

Key design points vs a naive port:
  - All GEMMs run in bf16 (full PE rate; fp32r is half rate on TRN2).
    Weights are pre-transposed and pre-cast to bf16 on the host.
  - BN1 stats are computed locally on EVERY core from a replicated fp8
    copy of the full x via the Gram matrix (mean/var of x@W1.T are a
    bilinear form of Gram(x)). This removes the first AllReduce entirely,
    so the cross-core rendezvous barrier (~100us of start skew) overlaps
    useful work instead of stalling the main pass.
  - h2 stays resident in SBUF in bf16 ([128, 8*6250] = 100KB/partition),
    eliminating the 51MB DRAM spill+reload between the BN2-stats pass and
    the normalize pass. Only BN2 stats need a (tiny, 8KB) AllReduce.
  - Elementwise work (BN+ReLU, evictions, sum-of-squares) is spread
    across scalar/vector/gpsimd so the PE stays the bottleneck.
"""

import os
import sys

sys.path.insert(0, "/opt/trn_rl_repo")

import numpy as np

NCORES = 8
N_TOTAL = 50000
R = N_TOTAL // NCORES  # 6250 rows per core
DIN = 128
H = 1024
HM = 256
C = 10
BN_EPS = 1e-5
CH = 512  # row-chunk (matmul moving dim)

NT = (N_TOTAL + 127) // 128  # 391 full-x row tiles -> pad to 392
NT += NT % 2
TD = DIN + 1  # 129: x tile plus a ones column (Gram + colsum in one matmul)

NRTT = (R + 127) // 128  # 49 row tiles per core
NFULL = R // 128  # 48 full tiles
CH_LIST = [(i * CH, min(CH, R - i * CH)) for i in range((R + CH - 1) // CH)]
NCH = len(CH_LIST)
if os.environ.get("CH_LIMIT"):
    CH_LIST = CH_LIST[: int(os.environ["CH_LIMIT"])]

_CACHE = {}


def _build(stage="full"):
    import concourse.bass as bass  # noqa: F401
    import concourse.tile as tile
    import concourse.mybir as mybir
    from concourse import bacc
    from concourse.masks import make_identity

    fp32 = mybir.dt.float32
    bf16 = mybir.dt.bfloat16
    fp8 = mybir.dt.float8e4
    AF = mybir.ActivationFunctionType
    ALU = mybir.AluOpType
    X = mybir.AxisListType.X

    nc = bacc.Bacc(num_devices=NCORES, debug=False)

    xT_d = nc.dram_tensor("xT", [DIN, R], bf16, kind="ExternalInput")
    xf8_d = nc.dram_tensor("xf8", [128, NT, TD], fp8, kind="ExternalInput")
    w1T_d = nc.dram_tensor("w1T", [DIN, H], bf16, kind="ExternalInput")
    w2T_d = nc.dram_tensor("w2T", [H, H], bf16, kind="ExternalInput")
    wl1T_d = nc.dram_tensor("wl1T", [H, HM], bf16, kind="ExternalInput")
    wl2T_d = nc.dram_tensor("wl2T", [HM, C], bf16, kind="ExternalInput")
    bl2_d = nc.dram_tensor("bl2", [1, C], bf16, kind="ExternalInput")
    # vg: [128, 34] f32: cols 0-7 g1, 8-15 be1, 16-23 g2, 24-31 be2, 32-33 bl1
    # (per-feature vectors laid out as [p, m] with feature = m*128 + p)
    vg_d = nc.dram_tensor("vg", [128, 34], fp32, kind="ExternalInput")
    out_d = nc.dram_tensor("out", [R, C], fp32, kind="ExternalOutput")

    RG = [list(range(NCORES))]

    with tile.TileContext(nc) as tc:
        with (
            tc.tile_pool(name="persist", bufs=1) as persist,
            tc.tile_pool(name="work", bufs=2) as work,
            tc.tile_pool(name="dram", bufs=1, space="DRAM") as dram,
        ):
            # ---------------- persistent tiles -----------------
            identity = persist.tile([128, 128], fp32, tag="identity", name="identity")
            make_identity(nc, identity[:])
            ones_col = persist.tile([128, 1], fp32, tag="ones", name="ones_col")
            nc.vector.memset(ones_col[:], 1.0)
            ones_row = persist.tile([1, CH], bf16, tag="onesr", name="ones_row")
            nc.vector.memset(ones_row[:], 1.0)

            xT = persist.tile([128, R], bf16, tag="xT", name="xT")
            w1T = persist.tile([128, H], bf16, tag="w1T", name="w1T")
            w2T = persist.tile([128, 8 * H], bf16, tag="w2T", name="w2T")
            wl1T = persist.tile([128, 8 * HM], bf16, tag="wl1T", name="wl1T")
            wl2T = persist.tile([128, 2 * C], bf16, tag="wl2T", name="wl2T")
            bl2 = persist.tile([1, C], bf16, tag="bl2", name="bl2")
            vg = persist.tile([128, 34], fp32, tag="vg", name="vg")

            bn1_scale = persist.tile([128, 8], fp32, tag="bn1s", name="bn1_scale")
            bn1_bias = persist.tile([128, 8], fp32, tag="bn1b", name="bn1_bias")
            bn2_scale = persist.tile([128, 8], fp32, tag="bn2s", name="bn2_scale")
            bn2_bias = persist.tile([128, 8], fp32, tag="bn2b", name="bn2_bias")

            sum_parts = persist.tile([128, 8 * NCH], fp32, tag="sump", name="sum_parts")
            sumsq_parts = persist.tile(
                [128, 8 * NCH], fp32, tag="sumq", name="sumsq_parts"
            )
            stats2_sb = persist.tile([128, 16], fp32, tag="st2", name="stats2_sb")
            stats2g = persist.tile([128, 16], fp32, tag="st2g", name="stats2g")

            rows_all = persist.tile([128, NRTT * C], fp32, tag="rows_all", name="rows_all")
            nc.vector.memset(rows_all[:], 0.0)
            e_all = persist.tile([128, NRTT * C], fp32, tag="e_all", name="e_all")
            res_all = persist.tile([128, NRTT * C], fp32, tag="res_all", name="res_all")
            sums_all = persist.tile([128, NRTT], fp32, tag="sums_all", name="sums_all")
            lse_all = persist.tile([128, NRTT], fp32, tag="lse_all", name="lse_all")

            cc2_in = dram.tile([128, 16], fp32, name="cc2_in")
            cc2_out = dram.tile([128, 16], fp32, name="cc2_out")

            # ============ startup: loads + replicated Gram -> BN1 stats ============
            with tc.tile_pool(name="boot", bufs=1) as boot, \
                 tc.tile_pool(name="pg", bufs=1, space="PSUM") as pg, \
                 tc.tile_pool(name="pv", bufs=2, space="PSUM") as pv, \
                 tc.tile_pool(name="psmall", bufs=4, space="PSUM") as psmall:
                xf8 = boot.tile([128, NT, TD], fp8, tag="xf8", name="xf8")
                # xf8 gates BN1 stats -> issue its pieces first, one per engine
                dma_eng = [nc.sync, nc.scalar, nc.gpsimd, nc.sync]
                NP = 4
                PT = NT // NP  # 98 tiles per piece
                for P in range(NP):
                    dma_eng[P].dma_start(
                        out=xf8[:, P * PT : (P + 1) * PT, :],
                        in_=xf8_d[:, P * PT : (P + 1) * PT, :],
                    )
                # remaining loads ride behind the xf8 pieces
                nc.sync.dma_start(out=xT[:], in_=xT_d[:])
                nc.scalar.dma_start(out=w1T[:], in_=w1T_d[:])
                nc.gpsimd.dma_start(
                    out=w2T[:], in_=w2T_d[:].rearrange("(k p) h -> p k h", p=128)
                )
                nc.scalar.dma_start(
                    out=wl1T[:], in_=wl1T_d[:].rearrange("(k p) h -> p k h", p=128)
                )
                nc.sync.dma_start(
                    out=wl2T[:], in_=wl2T_d[:].rearrange("(k p) c -> p k c", p=128)
                )
                nc.scalar.dma_start(out=bl2[:], in_=bl2_d[:])
                nc.gpsimd.dma_start(out=vg[:], in_=vg_d[:])

                # Gram of the full x: G[d1,d2] = sum_n x[n,d1] x[n,d2]; the
                # ones column makes out[:, 128] the column sums. Grouped per
                # DMA piece so matmuls start as soon as each piece lands.
                gram_ps = pg.tile([128, TD], fp32, tag="gram", name="gram_ps")
                for t in range(NT):
                    nc.tensor.matmul(
                        gram_ps[:],
                        lhsT=xf8[:, t, 0:DIN],
                        rhs=xf8[:, t, :],
                        start=(t == 0),
                        stop=(t == NT - 1),
                    )

                # BN1 stats from Gram:  mean = W1 @ (cs/N),
                # E[h1^2]_j = (w_j^T G w_j)/N = colsum(w1T .* (G @ W1^T))_j / N
                g_bf = boot.tile([128, DIN], bf16, tag="g_bf", name="g_bf")
                nc.scalar.copy(g_bf[:], gram_ps[:, 0:DIN])
                mean_f = work.tile([128, 1], fp32, tag="meanf", name="mean_f")
                nc.scalar.mul(mean_f[:], gram_ps[:, DIN : DIN + 1], 1.0 / N_TOTAL)
                mean_bf = boot.tile([128, 1], bf16, tag="meanbf", name="mean_bf")
                nc.scalar.copy(mean_bf[:], mean_f[:])

                V_sb = boot.tile([128, H], fp32, tag="Vsb", name="V_sb")
                for half in range(2):
                    mp = pv.tile([128, 512], fp32, tag="pv", name=f"mp{half}")
                    nc.tensor.matmul(
                        mp[:],
                        lhsT=g_bf[:],
                        rhs=w1T[:, half * 512 : (half + 1) * 512],
                        start=True,
                        stop=True,
                    )
                    nc.vector.tensor_mul(
                        V_sb[:, half * 512 : (half + 1) * 512],
                        w1T[:, half * 512 : (half + 1) * 512],
                        mp[:],
                    )
                E2 = work.tile([128, 8], fp32, tag="E2", name="E2")
                WXM = work.tile([128, 8], fp32, tag="WXM", name="WXM")
                for m in range(8):
                    sl = slice(m * 128, (m + 1) * 128)
                    e2_ps = psmall.tile([128, 1], fp32, tag="psm", name=f"e2{m}")
                    nc.tensor.matmul(
                        e2_ps[:], lhsT=V_sb[:, sl], rhs=ones_col[:],
                        start=True, stop=True,
                    )
                    nc.vector.tensor_copy(E2[:, m : m + 1], e2_ps[:])
                    wxm_ps = psmall.tile([128, 1], fp32, tag="psm", name=f"wxm{m}")
                    nc.tensor.matmul(
                        wxm_ps[:], lhsT=w1T[:, sl], rhs=mean_bf[:],
                        start=True, stop=True,
                    )
                    nc.vector.tensor_copy(WXM[:, m : m + 1], wxm_ps[:])

                # vectorized coeffs: scale = g1 / sqrt(E2/N - WXM^2 + eps)
                #                    bias  = be1 - WXM * scale
                var_t = work.tile([128, 8], fp32, tag="var1", name="var1")
                nc.vector.tensor_scalar_mul(var_t[:], E2[:], 1.0 / N_TOTAL)
                msq = work.tile([128, 8], fp32, tag="msq1", name="msq1")
                nc.vector.tensor_mul(msq[:], WXM[:], WXM[:])
                nc.vector.tensor_sub(var_t[:], var_t[:], msq[:])
                nc.vector.tensor_scalar_add(var_t[:], var_t[:], BN_EPS)
                sd = work.tile([128, 8], fp32, tag="sd1", name="sd1")
                nc.scalar.sqrt(sd[:], var_t[:])
                rstd = work.tile([128, 8], fp32, tag="rstd1", name="rstd1")
                nc.vector.reciprocal(rstd[:], sd[:])
                nc.vector.tensor_mul(bn1_scale[:], rstd[:], vg[:, 0:8])
                t2 = work.tile([128, 8], fp32, tag="t21", name="t21")
                nc.vector.tensor_mul(t2[:], WXM[:], bn1_scale[:])
                nc.vector.tensor_sub(bn1_bias[:], vg[:, 8:16], t2[:])

            if stage == "s1":
                dummy = work.tile([128, C], fp32, tag="dummy", name="dummy")
                nc.vector.tensor_copy(dummy[:, 0:8], bn1_scale[:])
                nc.vector.tensor_copy(dummy[:, 8:10], bn1_bias[:, 0:2])
                for r0 in range(0, R, 128):
                    rr = min(128, R - r0)
                    nc.sync.dma_start(out=out_d[r0 : r0 + rr, :], in_=dummy[:rr, :])
            else:
                _build_rest(
                    nc, tc, stage, mybir, fp32, bf16, AF, ALU, X,
                    persist, work, dram, identity, ones_row, bl2, vg,
                    w1T, w2T, wl1T, wl2T, xT,
                    bn1_scale, bn1_bias, bn2_scale, bn2_bias,
                    sum_parts, sumsq_parts, stats2_sb, stats2g,
                    rows_all, e_all, res_all, sums_all, lse_all,
                    cc2_in, cc2_out, out_d, RG,
                )

    nc.finalize()
    return nc


def _build_rest(
    nc, tc, stage, mybir, fp32, bf16, AF, ALU, X,
    persist, work, dram, identity, ones_row, bl2, vg,
    w1T, w2T, wl1T, wl2T, xT,
    bn1_scale, bn1_bias, bn2_scale, bn2_bias,
    sum_parts, sumsq_parts, stats2_sb, stats2g,
    rows_all, e_all, res_all, sums_all, lse_all,
    cc2_in, cc2_out, out_d, RG,
):
    with (
        tc.tile_pool(name="hpool", bufs=1) as hpool,
        tc.tile_pool(name="acts", bufs=1) as acts,
        tc.tile_pool(name="sqs", bufs=2) as sqs,
        tc.tile_pool(name="h3pool", bufs=2) as h3pool,
        tc.tile_pool(name="lgpool", bufs=2) as lgpool,
    ):
        Hbig = hpool.tile([128, 8 * R], bf16, tag="Hbig", name="Hbig")

        # ------------- pass B: L1 -> BN1+ReLU -> L2, h2 -> SBUF -------------
        with tc.tile_pool(name="ph1", bufs=3, space="PSUM") as ph1, \
             tc.tile_pool(name="ph2", bufs=3, space="PSUM") as ph2:
            for j, (c0, cc) in enumerate(CH_LIST):
                a1 = [
                    acts.tile([128, CH], bf16, tag=f"a1_{k}", name=f"a1_{j}_{k}")
                    for k in range(8)
                ]
                for m in range(8):
                    sl = slice(m * 128, (m + 1) * 128)
                    h1_ps = ph1.tile([128, CH], fp32, tag="ph1", name=f"h1ps{j}_{m}")
                    nc.tensor.matmul(
                        h1_ps[:, :cc],
                        lhsT=w1T[:, sl],
                        rhs=xT[:, c0 : c0 + cc],
                        start=True,
                        stop=True,
                    )
                    nc.scalar.activation(
                        a1[m][:, :cc],
                        h1_ps[:, :cc],
                        AF.Relu,
                        bias=bn1_bias[:, m : m + 1],
                        scale=bn1_scale[:, m : m + 1],
                    )
                for m in range(8):
                    h2_ps = ph2.tile([128, CH], fp32, tag="ph2", name=f"h2ps{j}_{m}")
                    for k in range(8):
                        nc.tensor.matmul(
                            h2_ps[:, :cc],
                            lhsT=w2T[:, k * H + m * 128 : k * H + (m + 1) * 128],
                            rhs=a1[k][:, :cc],
                            start=(k == 0),
                            stop=(k == 7),
                        )
                    Hs = Hbig[:, m * R + c0 : m * R + c0 + cc]
                    sacc = sum_parts[:, m * NCH + j : m * NCH + j + 1]
                    qacc = sumsq_parts[:, m * NCH + j : m * NCH + j + 1]
                    sq = sqs.tile([128, CH], bf16, tag=f"sq{m % 4}", name=f"sq{j}_{m}")
                    if m < 4:
                        # evict+sum and sum-of-squares both on scalar
                        nc.scalar.activation(
                            Hs, h2_ps[:, :cc], AF.Identity,
                            bias=0.0, scale=1.0, accum_out=sacc,
                        )
                        nc.scalar.activation(
                            sq[:, :cc], Hs, AF.Square,
                            bias=0.0, scale=1.0, accum_out=qacc,
                        )
                    else:
                        # evict+sum and sum-of-squares both on vector
                        nc.vector.tensor_scalar(
                            out=Hs, in0=h2_ps[:, :cc], scalar1=1.0, scalar2=0.0,
                            op0=ALU.mult, op1=ALU.add, accum_out=sacc,
                        )
                        nc.vector.scalar_tensor_tensor(
                            out=sq[:, :cc], in0=Hs, scalar=1.0, in1=Hs,
                            op0=ALU.mult, op1=ALU.mult, accum_out=qacc,
                        )

        # ---------------- BN2 statistics (the only collective) ----------------
        for m in range(8):
            eng = nc.vector
            eng.reduce_sum(
                stats2_sb[:, m : m + 1],
                sum_parts[:, m * NCH : (m + 1) * NCH],
                axis=X,
            )
            eng.reduce_sum(
                stats2_sb[:, 8 + m : 9 + m],
                sumsq_parts[:, m * NCH : (m + 1) * NCH],
                axis=X,
            )
        nc.sync.dma_start(out=cc2_in[:], in_=stats2_sb[:])
        nc.gpsimd.collective_compute(
            "AllReduce",
            mybir.AluOpType.add,
            replica_groups=RG,
            ins=[cc2_in[:].opt()],
            outs=[cc2_out[:].opt()],
        )
        nc.sync.dma_start(out=stats2g[:], in_=cc2_out[:])

        mean2 = work.tile([128, 8], fp32, tag="mean2", name="mean2")
        nc.vector.tensor_scalar_mul(mean2[:], stats2g[:, 0:8], 1.0 / N_TOTAL)
        var2 = work.tile([128, 8], fp32, tag="var2", name="var2")
        nc.vector.tensor_scalar_mul(var2[:], stats2g[:, 8:16], 1.0 / N_TOTAL)
        msq2 = work.tile([128, 8], fp32, tag="msq2", name="msq2")
        nc.vector.tensor_mul(msq2[:], mean2[:], mean2[:])
        nc.vector.tensor_sub(var2[:], var2[:], msq2[:])
        nc.vector.tensor_scalar_add(var2[:], var2[:], BN_EPS)
        sd2 = work.tile([128, 8], fp32, tag="sd2", name="sd2")
        nc.scalar.sqrt(sd2[:], var2[:])
        rstd2 = work.tile([128, 8], fp32, tag="rstd2", name="rstd2")
        nc.vector.reciprocal(rstd2[:], sd2[:])
        nc.vector.tensor_mul(bn2_scale[:], rstd2[:], vg[:, 16:24])
        t22 = work.tile([128, 8], fp32, tag="t22", name="t22")
        nc.vector.tensor_mul(t22[:], mean2[:], bn2_scale[:])
        nc.vector.tensor_sub(bn2_bias[:], vg[:, 24:32], t22[:])

        # -------- pass C: BN2+ReLU -> L3 -> L4 -> log_softmax --------
        with tc.tile_pool(name="ph3", bufs=2, space="PSUM") as ph3, \
             tc.tile_pool(name="plog", bufs=2, space="PSUM") as plog, \
             tc.tile_pool(name="ptr2", bufs=3, space="PSUM") as ptr2:
            for j, (c0, cc) in enumerate(CH_LIST):
                a2 = [
                    acts.tile([128, CH], bf16, tag=f"a1_{k}", name=f"a2_{j}_{k}")
                    for k in range(8)
                ]
                for k in range(8):
                    src = Hbig[:, k * R + c0 : k * R + c0 + cc]
                    if k < 4:
                        nc.scalar.activation(
                            a2[k][:, :cc], src, AF.Relu,
                            bias=bn2_bias[:, k : k + 1],
                            scale=bn2_scale[:, k : k + 1],
                        )
                    else:
                        eng = nc.vector if k < 6 else nc.gpsimd
                        tmp = sqs.tile(
                            [128, CH], bf16, tag=f"sq{k % 4}", name=f"af{j}_{k}"
                        )
                        eng.tensor_scalar(
                            out=tmp[:, :cc], in0=src,
                            scalar1=bn2_scale[:, k : k + 1],
                            scalar2=bn2_bias[:, k : k + 1],
                            op0=mybir.AluOpType.mult, op1=mybir.AluOpType.add,
                        )
                        eng.tensor_scalar_max(a2[k][:, :cc], tmp[:, :cc], 0.0)
                h3 = [
                    h3pool.tile([128, CH], bf16, tag=f"h3_{m3}", name=f"h3_{j}_{m3}")
                    for m3 in range(2)
                ]
                for m3 in range(2):
                    h3_ps = ph3.tile([128, CH], fp32, tag="ph3", name=f"h3ps{j}_{m3}")
                    for k in range(8):
                        nc.tensor.matmul(
                            h3_ps[:, :cc],
                            lhsT=wl1T[:, k * HM + m3 * 128 : k * HM + (m3 + 1) * 128],
                            rhs=a2[k][:, :cc],
                            start=(k == 0),
                            stop=(k == 7),
                        )
                    nc.scalar.activation(
                        h3[m3][:, :cc], h3_ps[:, :cc], AF.Relu,
                        bias=vg[:, 32 + m3 : 33 + m3], scale=1.0,
                    )
                lg_ps = plog.tile([C, CH], fp32, tag="plog", name=f"lg{j}")
                nc.tensor.matmul(
                    lg_ps[:, :cc],
                    lhsT=bl2[:],
                    rhs=ones_row[:, :cc],
                    start=True,
                    stop=False,
                )
                for k3 in range(2):
                    nc.tensor.matmul(
                        lg_ps[:, :cc],
                        lhsT=wl2T[:, k3 * C : (k3 + 1) * C],
                        rhs=h3[k3][:, :cc],
                        start=False,
                        stop=(k3 == 1),
                    )
                lg_sb = lgpool.tile([C, CH], fp32, tag="lg", name=f"lgs{j}")
                nc.vector.tensor_copy(lg_sb[:, :cc], lg_ps[:, :cc])
                # transpose logits to row-major and collect into rows_all
                nt = (cc + 127) // 128
                for t in range(nt):
                    rt0 = t * 128
                    rt = min(128, cc - rt0)
                    tg = (c0 + rt0) // 128
                    tp_ps = ptr2.tile([128, C], fp32, tag="ptr2", name=f"sm{j}_{t}")
                    nc.tensor.transpose(
                        tp_ps[:rt, :],
                        lg_sb[:, rt0 : rt0 + rt],
                        identity[:C, :C],
                    )
                    nc.vector.tensor_copy(
                        rows_all[:rt, tg * C : (tg + 1) * C], tp_ps[:rt, :]
                    )

            # ---- batched log_softmax over all row tiles ----
            # logits are O(10), so exp() without max-subtraction is safe in f32
            nc.scalar.activation(e_all[:], rows_all[:], AF.Exp)
            nc.vector.reduce_sum(
                sums_all[:],
                e_all[:].rearrange("p (t c) -> p t c", c=C),
                axis=X,
            )
            nc.scalar.activation(lse_all[:], sums_all[:], AF.Ln)
            nc.vector.tensor_sub(
                res_all[:].rearrange("p (t c) -> p t c", c=C),
                rows_all[:].rearrange("p (t c) -> p t c", c=C),
                lse_all[:].to_broadcast([128, NRTT, C]),
            )
            nc.sync.dma_start(
                out=out_d[: NFULL * 128].rearrange("(t p) c -> p t c", p=128),
                in_=res_all[:, : NFULL * C],
            )
            rtail = R - NFULL * 128
            if rtail:
                nc.sync.dma_start(
                    out=out_d[NFULL * 128 :],
                    in_=res_all[:rtail, NFULL * C :],
                )


def _get_nc():
    if "nc" not in _CACHE:
        _CACHE["nc"] = _build(os.environ.get("KERNEL_STAGE", "full"))
    return _CACHE["nc"]


def make_in_maps(inputs):
    import ml_dtypes

    BF = ml_dtypes.bfloat16
    F8 = ml_dtypes.float8_e4m3
    f32 = np.float32

    x = np.ascontiguousarray(np.asarray(inputs["x"]), dtype=f32)
    W1 = np.asarray(inputs["W1"], dtype=f32)
    W2 = np.asarray(inputs["W2"], dtype=f32)
    Wl1 = np.asarray(inputs["Wl1"], dtype=f32)
    Wl2 = np.asarray(inputs["Wl2"], dtype=f32)

    # full-x fp8 tiles [p, t, d]: x[t*128 + p, d], plus a ones column
    xpad = np.zeros((NT * 128, DIN), f32)
    xpad[:N_TOTAL] = x
    xf8 = np.empty((128, NT, TD), F8)
    xf8[:, :, :DIN] = xpad.reshape(NT, 128, DIN).transpose(1, 0, 2).astype(F8)
    xf8[:, :, DIN] = f32(1.0)

    w1T = np.ascontiguousarray(W1.T).astype(BF)
    w2T = np.ascontiguousarray(W2.T).astype(BF)
    wl1T = np.ascontiguousarray(Wl1.T).astype(BF)
    wl2T = np.ascontiguousarray(Wl2.T).astype(BF)
    bl2 = np.asarray(inputs["bl2"], f32).reshape(1, C).astype(BF)

    vg = np.zeros((128, 34), f32)
    vg[:, 0:8] = np.asarray(inputs["g1"], f32).reshape(8, 128).T
    vg[:, 8:16] = np.asarray(inputs["be1"], f32).reshape(8, 128).T
    vg[:, 16:24] = np.asarray(inputs["g2"], f32).reshape(8, 128).T
    vg[:, 24:32] = np.asarray(inputs["be2"], f32).reshape(8, 128).T
    vg[:, 32:34] = np.asarray(inputs["bl1"], f32).reshape(2, 128).T

    return [
        {
            "xT": np.ascontiguousarray(x[i * R : (i + 1) * R].T).astype(BF),
            "xf8": xf8,
            "w1T": w1T,
            "w2T": w2T,
            "wl1T": wl1T,
            "wl2T": wl2T,
            "bl2": bl2,
            "vg": vg,
        }
        for i in range(NCORES)
    ]


def kernel(**inputs):
    from concourse.bass_utils import run_bass_kernel_spmd

    nc = _get_nc()
    in_maps = make_in_maps(inputs)
    res = run_bass_kernel_spmd(nc, in_maps, core_ids=list(range(NCORES)))
    return np.concatenate([r["out"] for r in res.results], axis=0).astype(np.float32)


# revision 13
# speedup vs baseline: 1.2626x; 1.2626x over previous
"""Trainium2 Bass kernel for ChebyNet (K=1) forward pass.

ChebConv with K=1 reduces to a plain linear layer on the T0 (identity) term,
so edge_index / edge_weight never enter the math. The network is:

    h1 = x @ W1.T (+b1, dropped: BN cancels additive bias) -> BN -> ReLU
    h2 = h1 @ W2.T (+b2, dropped)                          -> BN -> ReLU
    h3 = relu(h2 @ Wl1.T + bl1)
    out = log_softmax(h3 @ Wl2.T + bl2, axis=1)

Sharding: nodes (N=50000) split across 8 NeuronCores (R=6250 rows each).

Design (vs a naive port):
  - All GEMMs in bf16; weights pre-transposed/pre-cast on the host.
  - BN1 stats come from the Gram matrix of the local x shard (fp8; stats
    only) via the bilinear identity E[h1^2]_j = w_j^T G w_j / N. The
    per-core partial sums are a tiny [128,16] AllReduce issued ~15us into
    the kernel, so the cross-core start-skew is absorbed while pass A runs.
  - Pass A computes L1 (stats-independent) into the big SBUF buffer while
    that AllReduce is in flight.
  - h2 stays resident in SBUF in bf16 ([128, 8*6250] = 100KB/partition):
    no DRAM spill. Only BN2 stats need a second (8KB) AllReduce, cheap
    because cores are already aligned by the first one.
  - Elementwise work is split scalar/vector (gpsimd only for pass-A
    evictions: its MAX path is ~25x slower than vector's, so it never
    touches ReLU).
"""

import os
import sys

sys.path.insert(0, "/opt/trn_rl_repo")

import numpy as np

NCORES = 8
N_TOTAL = 50000
R = N_TOTAL // NCORES  # 6250 rows per core
DIN = 128
H = 1024
HM = 256
C = 10
BN_EPS = 1e-5
CH = 512  # row-chunk (matmul moving dim)

NRTT = (R + 127) // 128  # 49 row tiles per core
NFULL = R // 128  # 48 full tiles
TD = DIN + 1  # 129: x tile plus a ones column (Gram + colsum in one matmul)
CH_LIST = [(i * CH, min(CH, R - i * CH)) for i in range((R + CH - 1) // CH)]
NCH = len(CH_LIST)
if os.environ.get("CH_LIMIT"):
    CH_LIST = CH_LIST[: int(os.environ["CH_LIMIT"])]

_CACHE = {}


def _build(stage="full"):
    import concourse.bass as bass  # noqa: F401
    import concourse.tile as tile
    import concourse.mybir as mybir
    from concourse import bacc
    from concourse.masks import make_identity

    fp32 = mybir.dt.float32
    bf16 = mybir.dt.bfloat16
    fp8 = mybir.dt.float8e4
    AF = mybir.ActivationFunctionType
    ALU = mybir.AluOpType
    X = mybir.AxisListType.X

    nc = bacc.Bacc(num_devices=NCORES, debug=False)

    xT_d = nc.dram_tensor("xT", [DIN, R], bf16, kind="ExternalInput")
    xf8_d = nc.dram_tensor("xf8", [128, NRTT, TD], fp8, kind="ExternalInput")
    w1T_d = nc.dram_tensor("w1T", [DIN, H], bf16, kind="ExternalInput")
    w2T_d = nc.dram_tensor("w2T", [H, H], bf16, kind="ExternalInput")
    wl1T_d = nc.dram_tensor("wl1T", [H, HM], bf16, kind="ExternalInput")
    wl2T_d = nc.dram_tensor("wl2T", [HM, C], bf16, kind="ExternalInput")
    bl2_d = nc.dram_tensor("bl2", [1, C], bf16, kind="ExternalInput")
    # vg: [128, 34] f32: cols 0-7 g1, 8-15 be1, 16-23 g2, 24-31 be2, 32-33 bl1
    # (per-feature vectors laid out as [p, m] with feature = m*128 + p)
    vg_d = nc.dram_tensor("vg", [128, 34], fp32, kind="ExternalInput")
    out_d = nc.dram_tensor("out", [R, C], fp32, kind="ExternalOutput")

    RG = [list(range(NCORES))]

    with tile.TileContext(nc) as tc:
        with (
            tc.tile_pool(name="persist", bufs=1) as persist,
            tc.tile_pool(name="work", bufs=2) as work,
            tc.tile_pool(name="dram", bufs=1, space="DRAM") as dram,
        ):
            # ---------------- persistent tiles -----------------
            identity = persist.tile([128, 128], fp32, tag="identity", name="identity")
            make_identity(nc, identity[:])
            ones_col = persist.tile([128, 1], fp32, tag="ones", name="ones_col")
            nc.vector.memset(ones_col[:], 1.0)
            ones_row = persist.tile([1, CH], bf16, tag="onesr", name="ones_row")
            nc.vector.memset(ones_row[:], 1.0)

            xT = persist.tile([128, R], bf16, tag="xT", name="xT")
            xf8 = persist.tile([128, NRTT, TD], fp8, tag="xf8", name="xf8")
            w1T = persist.tile([128, H], bf16, tag="w1T", name="w1T")
            w2T = persist.tile([128, 8 * H], bf16, tag="w2T", name="w2T")
            wl1T = persist.tile([128, 8 * HM], bf16, tag="wl1T", name="wl1T")
            wl2T = persist.tile([128, 2 * C], bf16, tag="wl2T", name="wl2T")
            bl2 = persist.tile([1, C], bf16, tag="bl2", name="bl2")
            vg = persist.tile([128, 34], fp32, tag="vg", name="vg")
            Hbig = persist.tile([128, 8 * R], bf16, tag="Hbig", name="Hbig")

            bn1_scale = persist.tile([128, 8], fp32, tag="bn1s", name="bn1_scale")
            bn1_bias = persist.tile([128, 8], fp32, tag="bn1b", name="bn1_bias")
            bn2_scale = persist.tile([128, 8], fp32, tag="bn2s", name="bn2_scale")
            bn2_bias = persist.tile([128, 8], fp32, tag="bn2b", name="bn2_bias")

            sum_parts = persist.tile([128, 8 * NCH], fp32, tag="sump", name="sum_parts")
            sumsq_parts = persist.tile(
                [128, 8 * NCH], fp32, tag="sumq", name="sumsq_parts"
            )
            stats1_sb = persist.tile([128, 16], fp32, tag="st1", name="stats1_sb")
            stats1g = persist.tile([128, 16], fp32, tag="st1g", name="stats1g")
            stats2_sb = persist.tile([128, 16], fp32, tag="st2", name="stats2_sb")
            stats2g = persist.tile([128, 16], fp32, tag="st2g", name="stats2g")

            rows_all = persist.tile([128, NRTT * C], fp32, tag="rows_all", name="rows_all")
            nc.vector.memset(rows_all[:], 0.0)
            e_all = persist.tile([128, NRTT * C], fp32, tag="e_all", name="e_all")
            res_all = persist.tile([128, NRTT * C], fp32, tag="res_all", name="res_all")
            sums_all = persist.tile([128, NRTT], fp32, tag="sums_all", name="sums_all")
            lse_all = persist.tile([128, NRTT], fp32, tag="lse_all", name="lse_all")

            cc1_in = dram.tile([128, 16], fp32, name="cc1_in")
            cc1_out = dram.tile([128, 16], fp32, name="cc1_out")
            cc2_in = dram.tile([128, 16], fp32, name="cc2_in")
            cc2_out = dram.tile([128, 16], fp32, name="cc2_out")

            # ---------------- loads ----------------
            # xf8 (gates Gram -> AllReduce trigger) and xT (gates pass A)
            # land first on separate queues; weights ride behind.
            nc.sync.dma_start(out=xf8[:], in_=xf8_d[:])
            half = R // 2
            nc.scalar.dma_start(out=xT[:, :half], in_=xT_d[:, :half])
            nc.gpsimd.dma_start(out=xT[:, half:], in_=xT_d[:, half:])
            nc.scalar.dma_start(out=w1T[:], in_=w1T_d[:])
            nc.gpsimd.dma_start(
                out=w2T[:], in_=w2T_d[:].rearrange("(k p) h -> p k h", p=128)
            )
            nc.gpsimd.dma_start(
                out=wl1T[:], in_=wl1T_d[:].rearrange("(k p) h -> p k h", p=128)
            )
            nc.scalar.dma_start(
                out=wl2T[:], in_=wl2T_d[:].rearrange("(k p) c -> p k c", p=128)
            )
            nc.scalar.dma_start(out=bl2[:], in_=bl2_d[:])
            nc.scalar.dma_start(out=vg[:], in_=vg_d[:])

            # ====== shard Gram -> local BN1 partial sums -> AllReduce ======
            with tc.tile_pool(name="boot", bufs=1) as boot:
              with tc.tile_pool(name="pg", bufs=1, space="PSUM") as pg, \
                   tc.tile_pool(name="pv", bufs=2, space="PSUM") as pv, \
                   tc.tile_pool(name="psmall", bufs=4, space="PSUM") as psmall:
                gram_ps = pg.tile([128, TD], fp32, tag="gram", name="gram_ps")
                for t in range(NRTT):
                    nc.tensor.matmul(
                        gram_ps[:],
                        lhsT=xf8[:, t, 0:DIN],
                        rhs=xf8[:, t, :],
                        start=(t == 0),
                        stop=(t == NRTT - 1),
                    )
                # local partials: s = W1 @ cs_local, q_j = w_j^T G_local w_j
                g_bf = boot.tile([128, DIN], bf16, tag="g_bf", name="g_bf")
                nc.scalar.copy(g_bf[:], gram_ps[:, 0:DIN])
                cs_bf = boot.tile([128, 1], bf16, tag="cs_bf", name="cs_bf")
                nc.scalar.copy(cs_bf[:], gram_ps[:, DIN : DIN + 1])

                V_sb = boot.tile([128, H], fp32, tag="Vsb", name="V_sb")
                for halfv in range(2):
                    mp = pv.tile([128, 512], fp32, tag="pv", name=f"mp{halfv}")
                    nc.tensor.matmul(
                        mp[:],
                        lhsT=g_bf[:],
                        rhs=w1T[:, halfv * 512 : (halfv + 1) * 512],
                        start=True,
                        stop=True,
                    )
                    nc.vector.tensor_mul(
                        V_sb[:, halfv * 512 : (halfv + 1) * 512],
                        w1T[:, halfv * 512 : (halfv + 1) * 512],
                        mp[:],
                    )
                for m in range(8):
                    sl = slice(m * 128, (m + 1) * 128)
                    s_ps = psmall.tile([128, 1], fp32, tag="psm", name=f"s{m}")
                    nc.tensor.matmul(
                        s_ps[:], lhsT=w1T[:, sl], rhs=cs_bf[:],
                        start=True, stop=True,
                    )
                    nc.vector.tensor_copy(stats1_sb[:, m : m + 1], s_ps[:])
                    q_ps = psmall.tile([128, 1], fp32, tag="psm", name=f"q{m}")
                    nc.tensor.matmul(
                        q_ps[:], lhsT=V_sb[:, sl], rhs=ones_col[:],
                        start=True, stop=True,
                    )
                    nc.vector.tensor_copy(stats1_sb[:, 8 + m : 9 + m], q_ps[:])

                nc.sync.dma_start(out=cc1_in[:], in_=stats1_sb[:])
                nc.gpsimd.collective_compute(
                    "AllReduce",
                    ALU.add,
                    replica_groups=RG,
                    ins=[cc1_in[:].opt()],
                    outs=[cc1_out[:].opt()],
                )

              # ------- pass A: L1 into Hbig (no BN yet), overlaps AR1 -------
              if True:
                with tc.tile_pool(name="ph1", bufs=4, space="PSUM") as ph1:
                    for j, (c0, cc) in enumerate(CH_LIST):
                        for m in range(8):
                            sl = slice(m * 128, (m + 1) * 128)
                            h1_ps = ph1.tile(
                                [128, CH], fp32, tag="ph1", name=f"h1ps{j}_{m}"
                            )
                            nc.tensor.matmul(
                                h1_ps[:, :cc],
                                lhsT=w1T[:, sl],
                                rhs=xT[:, c0 : c0 + cc],
                                start=True,
                                stop=True,
                            )
                            Hs = Hbig[:, m * R + c0 : m * R + c0 + cc]
                            if m < 4:
                                nc.scalar.activation(
                                    Hs, h1_ps[:, :cc], AF.Identity,
                                    bias=0.0, scale=1.0,
                                )
                            else:
                                nc.vector.tensor_scalar(
                                    out=Hs, in0=h1_ps[:, :cc],
                                    scalar1=1.0, scalar2=0.0,
                                    op0=ALU.mult, op1=ALU.add,
                                )

                # ------- BN1 coeffs from the AllReduced stats -------
                nc.sync.dma_start(out=stats1g[:], in_=cc1_out[:])
                WXM = work.tile([128, 8], fp32, tag="WXM", name="WXM")
                nc.vector.tensor_scalar_mul(WXM[:], stats1g[:, 0:8], 1.0 / N_TOTAL)
                var_t = work.tile([128, 8], fp32, tag="var1", name="var1")
                nc.vector.tensor_scalar_mul(var_t[:], stats1g[:, 8:16], 1.0 / N_TOTAL)
                msq = work.tile([128, 8], fp32, tag="msq1", name="msq1")
                nc.vector.tensor_mul(msq[:], WXM[:], WXM[:])
                nc.vector.tensor_sub(var_t[:], var_t[:], msq[:])
                nc.vector.tensor_scalar_add(var_t[:], var_t[:], BN_EPS)
                sd = work.tile([128, 8], fp32, tag="sd1", name="sd1")
                nc.scalar.sqrt(sd[:], var_t[:])
                rstd = work.tile([128, 8], fp32, tag="rstd1", name="rstd1")
                nc.vector.reciprocal(rstd[:], sd[:])
                nc.vector.tensor_mul(bn1_scale[:], rstd[:], vg[:, 0:8])
                t2 = work.tile([128, 8], fp32, tag="t21", name="t21")
                nc.vector.tensor_mul(t2[:], WXM[:], bn1_scale[:])
                nc.vector.tensor_sub(bn1_bias[:], vg[:, 8:16], t2[:])

            if stage == "s1":
                dummy = work.tile([128, C], fp32, tag="dummy", name="dummy")
                nc.vector.tensor_copy(dummy[:, 0:8], bn1_scale[:])
                nc.vector.tensor_copy(dummy[:, 8:10], bn1_bias[:, 0:2])
                for r0 in range(0, R, 128):
                    rr = min(128, R - r0)
                    nc.sync.dma_start(out=out_d[r0 : r0 + rr, :], in_=dummy[:rr, :])
            else:
                _build_rest(
                    nc, tc, stage, mybir, fp32, bf16, AF, ALU, X,
                    persist, work, dram, identity, ones_row, bl2, vg,
                    w1T, w2T, wl1T, wl2T, Hbig,
                    bn1_scale, bn1_bias, bn2_scale, bn2_bias,
                    sum_parts, sumsq_parts, stats2_sb, stats2g,
                    rows_all, e_all, res_all, sums_all, lse_all,
                    cc2_in, cc2_out, out_d, RG,
                )

    nc.finalize()
    return nc


def _build_rest(
    nc, tc, stage, mybir, fp32, bf16, AF, ALU, X,
    persist, work, dram, identity, ones_row, bl2, vg,
    w1T, w2T, wl1T, wl2T, Hbig,
    bn1_scale, bn1_bias, bn2_scale, bn2_bias,
    sum_parts, sumsq_parts, stats2_sb, stats2g,
    rows_all, e_all, res_all, sums_all, lse_all,
    cc2_in, cc2_out, out_d, RG,
):
    with (
        tc.tile_pool(name="acts", bufs=1) as acts,
        tc.tile_pool(name="sqs", bufs=2) as sqs,
        tc.tile_pool(name="h3pool", bufs=2) as h3pool,
        tc.tile_pool(name="lgpool", bufs=2) as lgpool,
    ):
        # ----- pass B: a1 = BN1+ReLU(h1 from Hbig); h2 = a1 @ W2.T -> Hbig -----
        with tc.tile_pool(name="ph2", bufs=3, space="PSUM") as ph2:
            for j, (c0, cc) in enumerate(CH_LIST):
                a1 = [
                    acts.tile([128, CH], bf16, tag=f"a1_{k}", name=f"a1_{j}_{k}")
                    for k in range(8)
                ]
                for k in range(8):
                    nc.scalar.activation(
                        a1[k][:, :cc],
                        Hbig[:, k * R + c0 : k * R + c0 + cc],
                        AF.Relu,
                        bias=bn1_bias[:, k : k + 1],
                        scale=bn1_scale[:, k : k + 1],
                    )
                for m in range(8):
                    h2_ps = ph2.tile([128, CH], fp32, tag="ph2", name=f"h2ps{j}_{m}")
                    for k in range(8):
                        nc.tensor.matmul(
                            h2_ps[:, :cc],
                            lhsT=w2T[:, k * H + m * 128 : k * H + (m + 1) * 128],
                            rhs=a1[k][:, :cc],
                            start=(k == 0),
                            stop=(k == 7),
                        )
                    Hs = Hbig[:, m * R + c0 : m * R + c0 + cc]
                    sacc = sum_parts[:, m * NCH + j : m * NCH + j + 1]
                    qacc = sumsq_parts[:, m * NCH + j : m * NCH + j + 1]
                    sq = sqs.tile([128, CH], bf16, tag=f"sq{m % 4}", name=f"sq{j}_{m}")
                    if m < 4:
                        # evict+sum and sum-of-squares both on scalar
                        nc.scalar.activation(
                            Hs, h2_ps[:, :cc], AF.Identity,
                            bias=0.0, scale=1.0, accum_out=sacc,
                        )
                        nc.scalar.activation(
                            sq[:, :cc], Hs, AF.Square,
                            bias=0.0, scale=1.0, accum_out=qacc,
                        )
                    else:
                        # evict+sum and sum-of-squares both on vector
                        nc.vector.tensor_scalar(
                            out=Hs, in0=h2_ps[:, :cc], scalar1=1.0, scalar2=0.0,
                            op0=ALU.mult, op1=ALU.add, accum_out=sacc,
                        )
                        nc.vector.scalar_tensor_tensor(
                            out=sq[:, :cc], in0=Hs, scalar=1.0, in1=Hs,
                            op0=ALU.mult, op1=ALU.mult, accum_out=qacc,
                        )

        # ---------------- BN2 statistics (second tiny collective) ----------------
        for m in range(8):
            nc.vector.reduce_sum(
                stats2_sb[:, m : m + 1],
                sum_parts[:, m * NCH : (m + 1) * NCH],
                axis=X,
            )
            nc.vector.reduce_sum(
                stats2_sb[:, 8 + m : 9 + m],
                sumsq_parts[:, m * NCH : (m + 1) * NCH],
                axis=X,
            )
        nc.sync.dma_start(out=cc2_in[:], in_=stats2_sb[:])
        nc.gpsimd.collective_compute(
            "AllReduce",
            mybir.AluOpType.add,
            replica_groups=RG,
            ins=[cc2_in[:].opt()],
            outs=[cc2_out[:].opt()],
        )
        nc.sync.dma_start(out=stats2g[:], in_=cc2_out[:])

        mean2 = work.tile([128, 8], fp32, tag="mean2", name="mean2")
        nc.vector.tensor_scalar_mul(mean2[:], stats2g[:, 0:8], 1.0 / N_TOTAL)
        var2 = work.tile([128, 8], fp32, tag="var2", name="var2")
        nc.vector.tensor_scalar_mul(var2[:], stats2g[:, 8:16], 1.0 / N_TOTAL)
        msq2 = work.tile([128, 8], fp32, tag="msq2", name="msq2")
        nc.vector.tensor_mul(msq2[:], mean2[:], mean2[:])
        nc.vector.tensor_sub(var2[:], var2[:], msq2[:])
        nc.vector.tensor_scalar_add(var2[:], var2[:], BN_EPS)
        sd2 = work.tile([128, 8], fp32, tag="sd2", name="sd2")
        nc.scalar.sqrt(sd2[:], var2[:])
        rstd2 = work.tile([128, 8], fp32, tag="rstd2", name="rstd2")
        nc.vector.reciprocal(rstd2[:], sd2[:])
        nc.vector.tensor_mul(bn2_scale[:], rstd2[:], vg[:, 16:24])
        t22 = work.tile([128, 8], fp32, tag="t22", name="t22")
        nc.vector.tensor_mul(t22[:], mean2[:], bn2_scale[:])
        nc.vector.tensor_sub(bn2_bias[:], vg[:, 24:32], t22[:])

        # -------- pass C: BN2+ReLU -> L3 -> L4 -> log_softmax --------
        with tc.tile_pool(name="ph3", bufs=2, space="PSUM") as ph3, \
             tc.tile_pool(name="plog", bufs=2, space="PSUM") as plog, \
             tc.tile_pool(name="ptr2", bufs=3, space="PSUM") as ptr2:
            for j, (c0, cc) in enumerate(CH_LIST):
                a2 = [
                    acts.tile([128, CH], bf16, tag=f"a1_{k}", name=f"a2_{j}_{k}")
                    for k in range(8)
                ]
                for k in range(8):
                    nc.scalar.activation(
                        a2[k][:, :cc],
                        Hbig[:, k * R + c0 : k * R + c0 + cc],
                        AF.Relu,
                        bias=bn2_bias[:, k : k + 1],
                        scale=bn2_scale[:, k : k + 1],
                    )
                h3 = [
                    h3pool.tile([128, CH], bf16, tag=f"h3_{m3}", name=f"h3_{j}_{m3}")
                    for m3 in range(2)
                ]
                for m3 in range(2):
                    h3_ps = ph3.tile([128, CH], fp32, tag="ph3", name=f"h3ps{j}_{m3}")
                    for k in range(8):
                        nc.tensor.matmul(
                            h3_ps[:, :cc],
                            lhsT=wl1T[:, k * HM + m3 * 128 : k * HM + (m3 + 1) * 128],
                            rhs=a2[k][:, :cc],
                            start=(k == 0),
                            stop=(k == 7),
                        )
                    nc.scalar.activation(
                        h3[m3][:, :cc], h3_ps[:, :cc], AF.Relu,
                        bias=vg[:, 32 + m3 : 33 + m3], scale=1.0,
                    )
                lg_ps = plog.tile([C, CH], fp32, tag="plog", name=f"lg{j}")
                nc.tensor.matmul(
                    lg_ps[:, :cc],
                    lhsT=bl2[:],
                    rhs=ones_row[:, :cc],
                    start=True,
                    stop=False,
                )
                for k3 in range(2):
                    nc.tensor.matmul(
                        lg_ps[:, :cc],
                        lhsT=wl2T[:, k3 * C : (k3 + 1) * C],
                        rhs=h3[k3][:, :cc],
                        start=False,
                        stop=(k3 == 1),
                    )
                lg_sb = lgpool.tile([C, CH], fp32, tag="lg", name=f"lgs{j}")
                nc.vector.tensor_copy(lg_sb[:, :cc], lg_ps[:, :cc])
                # transpose logits to row-major and collect into rows_all
                nt = (cc + 127) // 128
                for t in range(nt):
                    rt0 = t * 128
                    rt = min(128, cc - rt0)
                    tg = (c0 + rt0) // 128
                    tp_ps = ptr2.tile([128, C], fp32, tag="ptr2", name=f"sm{j}_{t}")
                    nc.tensor.transpose(
                        tp_ps[:rt, :],
                        lg_sb[:, rt0 : rt0 + rt],
                        identity[:C, :C],
                    )
                    nc.vector.tensor_copy(
                        rows_all[:rt, tg * C : (tg + 1) * C], tp_ps[:rt, :]
                    )

            # ---- batched log_softmax over all row tiles ----
            # logits are O(10), so exp() without max-subtraction is safe in f32
            nc.scalar.activation(e_all[:], rows_all[:], AF.Exp)
            nc.vector.reduce_sum(
                sums_all[:],
                e_all[:].rearrange("p (t c) -> p t c", c=C),
                axis=X,
            )
            nc.scalar.activation(lse_all[:], sums_all[:], AF.Ln)
            nc.vector.tensor_sub(
                res_all[:].rearrange("p (t c) -> p t c", c=C),
                rows_all[:].rearrange("p (t c) -> p t c", c=C),
                lse_all[:].to_broadcast([128, NRTT, C]),
            )
            nc.sync.dma_start(
                out=out_d[: NFULL * 128].rearrange("(t p) c -> p t c", p=128),
                in_=res_all[:, : NFULL * C],
            )
            rtail = R - NFULL * 128
            if rtail:
                nc.sync.dma_start(
                    out=out_d[NFULL * 128 :],
                    in_=res_all[:rtail, NFULL * C :],
                )


def _get_nc():
    if "nc" not in _CACHE:
        _CACHE["nc"] = _build(os.environ.get("KERNEL_STAGE", "full"))
    return _CACHE["nc"]


def make_in_maps(inputs):
    import ml_dtypes

    BF = ml_dtypes.bfloat16
    F8 = ml_dtypes.float8_e4m3
    f32 = np.float32

    x = np.ascontiguousarray(np.asarray(inputs["x"]), dtype=f32)
    W1 = np.asarray(inputs["W1"], dtype=f32)
    W2 = np.asarray(inputs["W2"], dtype=f32)
    Wl1 = np.asarray(inputs["Wl1"], dtype=f32)
    Wl2 = np.asarray(inputs["Wl2"], dtype=f32)

    w1T = np.ascontiguousarray(W1.T).astype(BF)
    w2T = np.ascontiguousarray(W2.T).astype(BF)
    wl1T = np.ascontiguousarray(Wl1.T).astype(BF)
    wl2T = np.ascontiguousarray(Wl2.T).astype(BF)
    bl2 = np.asarray(inputs["bl2"], f32).reshape(1, C).astype(BF)

    vg = np.zeros((128, 34), f32)
    vg[:, 0:8] = np.asarray(inputs["g1"], f32).reshape(8, 128).T
    vg[:, 8:16] = np.asarray(inputs["be1"], f32).reshape(8, 128).T
    vg[:, 16:24] = np.asarray(inputs["g2"], f32).reshape(8, 128).T
    vg[:, 24:32] = np.asarray(inputs["be2"], f32).reshape(8, 128).T
    vg[:, 32:34] = np.asarray(inputs["bl1"], f32).reshape(2, 128).T

    # per-core fp8 shard tiles [p, t, d]: x[i*R + t*128 + p, d] + ones col
    maps = []
    for i in range(NCORES):
        xs = x[i * R : (i + 1) * R]
        xpad = np.zeros((NRTT * 128, DIN), f32)
        xpad[:R] = xs
        xf8 = np.empty((128, NRTT, TD), F8)
        xf8[:, :, :DIN] = xpad.reshape(NRTT, 128, DIN).transpose(1, 0, 2).astype(F8)
        xf8[:, :, DIN] = f32(1.0)
        maps.append(
            {
                "xT": np.ascontiguousarray(xs.T).astype(BF),
                "xf8": xf8,
                "w1T": w1T,
                "w2T": w2T,
                "wl1T": wl1T,
                "wl2T": wl2T,
                "bl2": bl2,
                "vg": vg,
            }
        )
    return maps


def kernel(**inputs):
    from concourse.bass_utils import run_bass_kernel_spmd

    nc = _get_nc()
    in_maps = make_in_maps(inputs)
    res = run_bass_kernel_spmd(nc, in_maps, core_ids=list(range(NCORES)))
    return np.concatenate([r["out"] for r in res.results], axis=0).astype(np.float32)


# revision 15
# speedup vs baseline: 1.2895x; 1.0214x over previous
"""Trainium2 Bass kernel for ChebyNet (K=1) forward pass.

ChebConv with K=1 reduces to a plain linear layer on the T0 (identity) term,
so edge_index / edge_weight never enter the math. The network is:

    h1 = x @ W1.T (+b1, dropped: BN cancels additive bias) -> BN -> ReLU
    h2 = h1 @ W2.T (+b2, dropped)                          -> BN -> ReLU
    h3 = relu(h2 @ Wl1.T + bl1)
    out = log_softmax(h3 @ Wl2.T + bl2, axis=1)

Sharding: nodes (N=50000) split across 8 NeuronCores (R=6250 rows each).

Key design points vs a naive port:
  - All GEMMs run in bf16 (full PE rate; fp32r is half rate on TRN2).
    Weights are pre-transposed and pre-cast to bf16 on the host.
  - BN1 stats are computed locally on EVERY core from a replicated fp8
    copy of the full x via the Gram matrix (mean/var of x@W1.T are a
    bilinear form of Gram(x)). This removes the first AllReduce entirely,
    so the cross-core rendezvous barrier (~100us of start skew) overlaps
    useful work instead of stalling the main pass.
  - h2 stays resident in SBUF in bf16 ([128, 8*6250] = 100KB/partition),
    eliminating the 51MB DRAM spill+reload between the BN2-stats pass and
    the normalize pass. Only BN2 stats need a (tiny, 8KB) AllReduce.
  - Elementwise work (BN+ReLU, evictions, sum-of-squares) is spread
    across scalar/vector/gpsimd so the PE stays the bottleneck.
"""

import os
import sys

sys.path.insert(0, "/opt/trn_rl_repo")

import numpy as np

NCORES = 8
N_TOTAL = 50000
R = N_TOTAL // NCORES  # 6250 rows per core
DIN = 128
H = 1024
HM = 256
C = 10
BN_EPS = 1e-5
CH = 512  # row-chunk (matmul moving dim)

NT = (N_TOTAL + 127) // 128  # 391 full-x row tiles -> pad to 392
NT += NT % 2
TD = DIN + 1  # 129: x tile plus a ones column (Gram + colsum in one matmul)

NRTT = (R + 127) // 128  # 49 row tiles per core
NFULL = R // 128  # 48 full tiles
CH_LIST = [(i * CH, min(CH, R - i * CH)) for i in range((R + CH - 1) // CH)]
NCH = len(CH_LIST)
if os.environ.get("CH_LIMIT"):
    CH_LIST = CH_LIST[: int(os.environ["CH_LIMIT"])]

_CACHE = {}


def _build(stage="full"):
    import concourse.bass as bass  # noqa: F401
    import concourse.tile as tile
    import concourse.mybir as mybir
    from concourse import bacc
    from concourse.masks import make_identity

    fp32 = mybir.dt.float32
    bf16 = mybir.dt.bfloat16
    fp8 = mybir.dt.float8e4
    AF = mybir.ActivationFunctionType
    ALU = mybir.AluOpType
    X = mybir.AxisListType.X

    nc = bacc.Bacc(num_devices=NCORES, debug=False)

    xT_d = nc.dram_tensor("xT", [DIN, R], bf16, kind="ExternalInput")
    xf8_d = nc.dram_tensor("xf8", [128, NT, TD], fp8, kind="ExternalInput")
    w1T_d = nc.dram_tensor("w1T", [DIN, H], bf16, kind="ExternalInput")
    w2T_d = nc.dram_tensor("w2T", [H, H], bf16, kind="ExternalInput")
    wl1T_d = nc.dram_tensor("wl1T", [H, HM], bf16, kind="ExternalInput")
    wl2T_d = nc.dram_tensor("wl2T", [HM, C], bf16, kind="ExternalInput")
    bl2_d = nc.dram_tensor("bl2", [1, C], bf16, kind="ExternalInput")
    # vg: [128, 34] f32: cols 0-7 g1, 8-15 be1, 16-23 g2, 24-31 be2, 32-33 bl1
    # (per-feature vectors laid out as [p, m] with feature = m*128 + p)
    vg_d = nc.dram_tensor("vg", [128, 34], fp32, kind="ExternalInput")
    out_d = nc.dram_tensor("out", [R, C], fp32, kind="ExternalOutput")

    RG = [list(range(NCORES))]

    with tile.TileContext(nc) as tc:
        with (
            tc.tile_pool(name="persist", bufs=1) as persist,
            tc.tile_pool(name="work", bufs=2) as work,
            tc.tile_pool(name="dram", bufs=1, space="DRAM") as dram,
        ):
            # ---------------- persistent tiles -----------------
            identity = persist.tile([128, 128], fp32, tag="identity", name="identity")
            make_identity(nc, identity[:])
            ones_col = persist.tile([128, 1], fp32, tag="ones", name="ones_col")
            nc.vector.memset(ones_col[:], 1.0)
            ones_row = persist.tile([1, CH], bf16, tag="onesr", name="ones_row")
            nc.vector.memset(ones_row[:], 1.0)

            xT = persist.tile([128, R], bf16, tag="xT", name="xT")
            w1T = persist.tile([128, H], bf16, tag="w1T", name="w1T")
            w2T = persist.tile([128, 8 * H], bf16, tag="w2T", name="w2T")
            wl1T = persist.tile([128, 8 * HM], bf16, tag="wl1T", name="wl1T")
            wl2T = persist.tile([128, 2 * C], bf16, tag="wl2T", name="wl2T")
            bl2 = persist.tile([1, C], bf16, tag="bl2", name="bl2")
            vg = persist.tile([128, 34], fp32, tag="vg", name="vg")

            bn1_scale = persist.tile([128, 8], fp32, tag="bn1s", name="bn1_scale")
            bn1_bias = persist.tile([128, 8], fp32, tag="bn1b", name="bn1_bias")
            bn2_scale = persist.tile([128, 8], fp32, tag="bn2s", name="bn2_scale")
            bn2_bias = persist.tile([128, 8], fp32, tag="bn2b", name="bn2_bias")

            sum_parts = persist.tile([128, 8 * NCH], fp32, tag="sump", name="sum_parts")
            sumsq_parts = persist.tile(
                [128, 8 * NCH], fp32, tag="sumq", name="sumsq_parts"
            )
            stats2_sb = persist.tile([128, 16], fp32, tag="st2", name="stats2_sb")
            stats2g = persist.tile([128, 16], fp32, tag="st2g", name="stats2g")

            rows_all = persist.tile([128, NRTT * C], fp32, tag="rows_all", name="rows_all")
            nc.vector.memset(rows_all[:], 0.0)
            e_all = persist.tile([128, NRTT * C], fp32, tag="e_all", name="e_all")
            res_all = persist.tile([128, NRTT * C], fp32, tag="res_all", name="res_all")
            sums_all = persist.tile([128, NRTT], fp32, tag="sums_all", name="sums_all")
            lse_all = persist.tile([128, NRTT], fp32, tag="lse_all", name="lse_all")

            cc0_in = dram.tile([128, 16], fp32, name="cc0_in")
            cc0_out = dram.tile([128, 16], fp32, name="cc0_out")
            cc2_in = dram.tile([128, 16], fp32, name="cc2_in")
            cc2_out = dram.tile([128, 16], fp32, name="cc2_out")

            # skew-eater: a dummy AllReduce launched immediately; its result
            # is consumed mid-pass-B, so the cross-core start skew is absorbed
            # while compute runs instead of stalling the BN2 AllReduce.
            nc.vector.memset(stats2_sb[:], 0.0)
            nc.sync.dma_start(out=cc0_in[:], in_=stats2_sb[:])
            nc.gpsimd.collective_compute(
                "AllReduce",
                ALU.add,
                replica_groups=RG,
                ins=[cc0_in[:].opt()],
                outs=[cc0_out[:].opt()],
            )

            # ============ startup: loads + replicated Gram -> BN1 stats ============
            with tc.tile_pool(name="boot", bufs=1) as boot, \
                 tc.tile_pool(name="pg", bufs=1, space="PSUM") as pg, \
                 tc.tile_pool(name="pv", bufs=2, space="PSUM") as pv, \
                 tc.tile_pool(name="psmall", bufs=4, space="PSUM") as psmall:
                xf8 = boot.tile([128, NT, TD], fp8, tag="xf8", name="xf8")
                # xf8 gates BN1 stats -> issue its pieces first, one per engine
                dma_eng = [nc.sync, nc.scalar, nc.gpsimd, nc.sync]
                NP = 4
                PT = NT // NP  # 98 tiles per piece
                for P in range(NP):
                    dma_eng[P].dma_start(
                        out=xf8[:, P * PT : (P + 1) * PT, :],
                        in_=xf8_d[:, P * PT : (P + 1) * PT, :],
                    )
                # remaining loads ride behind the xf8 pieces
                nc.sync.dma_start(out=xT[:], in_=xT_d[:])
                nc.scalar.dma_start(out=w1T[:], in_=w1T_d[:])
                nc.gpsimd.dma_start(
                    out=w2T[:], in_=w2T_d[:].rearrange("(k p) h -> p k h", p=128)
                )
                nc.scalar.dma_start(
                    out=wl1T[:], in_=wl1T_d[:].rearrange("(k p) h -> p k h", p=128)
                )
                nc.sync.dma_start(
                    out=wl2T[:], in_=wl2T_d[:].rearrange("(k p) c -> p k c", p=128)
                )
                nc.scalar.dma_start(out=bl2[:], in_=bl2_d[:])
                nc.gpsimd.dma_start(out=vg[:], in_=vg_d[:])

                # Gram of the full x: G[d1,d2] = sum_n x[n,d1] x[n,d2]; the
                # ones column makes out[:, 128] the column sums. Grouped per
                # DMA piece so matmuls start as soon as each piece lands.
                gram_ps = pg.tile([128, TD], fp32, tag="gram", name="gram_ps")
                for t in range(NT):
                    nc.tensor.matmul(
                        gram_ps[:],
                        lhsT=xf8[:, t, 0:DIN],
                        rhs=xf8[:, t, :],
                        start=(t == 0),
                        stop=(t == NT - 1),
                    )

                # BN1 stats from Gram:  mean = W1 @ (cs/N),
                # E[h1^2]_j = (w_j^T G w_j)/N = colsum(w1T .* (G @ W1^T))_j / N
                g_bf = boot.tile([128, DIN], bf16, tag="g_bf", name="g_bf")
                nc.scalar.copy(g_bf[:], gram_ps[:, 0:DIN])
                mean_f = work.tile([128, 1], fp32, tag="meanf", name="mean_f")
                nc.scalar.mul(mean_f[:], gram_ps[:, DIN : DIN + 1], 1.0 / N_TOTAL)
                mean_bf = boot.tile([128, 1], bf16, tag="meanbf", name="mean_bf")
                nc.scalar.copy(mean_bf[:], mean_f[:])

                V_sb = boot.tile([128, H], fp32, tag="Vsb", name="V_sb")
                for half in range(2):
                    mp = pv.tile([128, 512], fp32, tag="pv", name=f"mp{half}")
                    nc.tensor.matmul(
                        mp[:],
                        lhsT=g_bf[:],
                        rhs=w1T[:, half * 512 : (half + 1) * 512],
                        start=True,
                        stop=True,
                    )
                    nc.vector.tensor_mul(
                        V_sb[:, half * 512 : (half + 1) * 512],
                        w1T[:, half * 512 : (half + 1) * 512],
                        mp[:],
                    )
                E2 = work.tile([128, 8], fp32, tag="E2", name="E2")
                WXM = work.tile([128, 8], fp32, tag="WXM", name="WXM")
                for m in range(8):
                    sl = slice(m * 128, (m + 1) * 128)
                    e2_ps = psmall.tile([128, 1], fp32, tag="psm", name=f"e2{m}")
                    nc.tensor.matmul(
                        e2_ps[:], lhsT=V_sb[:, sl], rhs=ones_col[:],
                        start=True, stop=True,
                    )
                    nc.vector.tensor_copy(E2[:, m : m + 1], e2_ps[:])
                    wxm_ps = psmall.tile([128, 1], fp32, tag="psm", name=f"wxm{m}")
                    nc.tensor.matmul(
                        wxm_ps[:], lhsT=w1T[:, sl], rhs=mean_bf[:],
                        start=True, stop=True,
                    )
                    nc.vector.tensor_copy(WXM[:, m : m + 1], wxm_ps[:])

                # vectorized coeffs: scale = g1 / sqrt(E2/N - WXM^2 + eps)
                #                    bias  = be1 - WXM * scale
                var_t = work.tile([128, 8], fp32, tag="var1", name="var1")
                nc.vector.tensor_scalar_mul(var_t[:], E2[:], 1.0 / N_TOTAL)
                msq = work.tile([128, 8], fp32, tag="msq1", name="msq1")
                nc.vector.tensor_mul(msq[:], WXM[:], WXM[:])
                nc.vector.tensor_sub(var_t[:], var_t[:], msq[:])
                nc.vector.tensor_scalar_add(var_t[:], var_t[:], BN_EPS)
                sd = work.tile([128, 8], fp32, tag="sd1", name="sd1")
                nc.scalar.sqrt(sd[:], var_t[:])
                rstd = work.tile([128, 8], fp32, tag="rstd1", name="rstd1")
                nc.vector.reciprocal(rstd[:], sd[:])
                nc.vector.tensor_mul(bn1_scale[:], rstd[:], vg[:, 0:8])
                t2 = work.tile([128, 8], fp32, tag="t21", name="t21")
                nc.vector.tensor_mul(t2[:], WXM[:], bn1_scale[:])
                nc.vector.tensor_sub(bn1_bias[:], vg[:, 8:16], t2[:])

            if stage == "s1":
                dummy = work.tile([128, C], fp32, tag="dummy", name="dummy")
                nc.vector.tensor_copy(dummy[:, 0:8], bn1_scale[:])
                nc.vector.tensor_copy(dummy[:, 8:10], bn1_bias[:, 0:2])
                for r0 in range(0, R, 128):
                    rr = min(128, R - r0)
                    nc.sync.dma_start(out=out_d[r0 : r0 + rr, :], in_=dummy[:rr, :])
            else:
                _build_rest(
                    nc, tc, stage, mybir, fp32, bf16, AF, ALU, X,
                    persist, work, dram, identity, ones_row, bl2, vg,
                    w1T, w2T, wl1T, wl2T, xT,
                    bn1_scale, bn1_bias, bn2_scale, bn2_bias,
                    sum_parts, sumsq_parts, stats2_sb, stats2g,
                    rows_all, e_all, res_all, sums_all, lse_all,
                    cc0_out, cc2_in, cc2_out, out_d, RG,
                )

    nc.finalize()
    return nc


def _build_rest(
    nc, tc, stage, mybir, fp32, bf16, AF, ALU, X,
    persist, work, dram, identity, ones_row, bl2, vg,
    w1T, w2T, wl1T, wl2T, xT,
    bn1_scale, bn1_bias, bn2_scale, bn2_bias,
    sum_parts, sumsq_parts, stats2_sb, stats2g,
    rows_all, e_all, res_all, sums_all, lse_all,
    cc0_out, cc2_in, cc2_out, out_d, RG,
):
    with (
        tc.tile_pool(name="hpool", bufs=1) as hpool,
        tc.tile_pool(name="acts", bufs=1) as acts,
        tc.tile_pool(name="sqs", bufs=2) as sqs,
        tc.tile_pool(name="h3pool", bufs=2) as h3pool,
        tc.tile_pool(name="lgpool", bufs=2) as lgpool,
    ):
        Hbig = hpool.tile([128, 8 * R], bf16, tag="Hbig", name="Hbig")

        # ------------- pass B: L1 -> BN1+ReLU -> L2, h2 -> SBUF -------------
        skew_sb = work.tile([128, 16], fp32, tag="skew", name="skew_sb")
        skew_c = work.tile([128, 1], fp32, tag="skewc", name="skew_c")
        with tc.tile_pool(name="ph1", bufs=3, space="PSUM") as ph1, \
             tc.tile_pool(name="ph2", bufs=3, space="PSUM") as ph2:
            for j, (c0, cc) in enumerate(CH_LIST):
                if j == 5:
                    nc.sync.dma_start(out=skew_sb[:], in_=cc0_out[:])
                    nc.vector.tensor_copy(skew_c[:], skew_sb[:, 0:1])
                a1 = [
                    acts.tile([128, CH], bf16, tag=f"a1_{k}", name=f"a1_{j}_{k}")
                    for k in range(8)
                ]
                for m in range(8):
                    sl = slice(m * 128, (m + 1) * 128)
                    h1_ps = ph1.tile([128, CH], fp32, tag="ph1", name=f"h1ps{j}_{m}")
                    nc.tensor.matmul(
                        h1_ps[:, :cc],
                        lhsT=w1T[:, sl],
                        rhs=xT[:, c0 : c0 + cc],
                        start=True,
                        stop=True,
                    )
                    nc.scalar.activation(
                        a1[m][:, :cc],
                        h1_ps[:, :cc],
                        AF.Relu,
                        bias=bn1_bias[:, m : m + 1],
                        scale=bn1_scale[:, m : m + 1],
                    )
                for m in range(8):
                    h2_ps = ph2.tile([128, CH], fp32, tag="ph2", name=f"h2ps{j}_{m}")
                    for k in range(8):
                        nc.tensor.matmul(
                            h2_ps[:, :cc],
                            lhsT=w2T[:, k * H + m * 128 : k * H + (m + 1) * 128],
                            rhs=a1[k][:, :cc],
                            start=(k == 0),
                            stop=(k == 7),
                        )
                    Hs = Hbig[:, m * R + c0 : m * R + c0 + cc]
                    sacc = sum_parts[:, m * NCH + j : m * NCH + j + 1]
                    qacc = sumsq_parts[:, m * NCH + j : m * NCH + j + 1]
                    sq = sqs.tile([128, CH], bf16, tag=f"sq{m % 4}", name=f"sq{j}_{m}")
                    if m < 4:
                        # evict+sum and sum-of-squares both on scalar
                        nc.scalar.activation(
                            Hs, h2_ps[:, :cc], AF.Identity,
                            bias=0.0, scale=1.0, accum_out=sacc,
                        )
                        nc.scalar.activation(
                            sq[:, :cc], Hs, AF.Square,
                            bias=0.0, scale=1.0, accum_out=qacc,
                        )
                    else:
                        # evict+sum and sum-of-squares both on vector
                        nc.vector.tensor_scalar(
                            out=Hs, in0=h2_ps[:, :cc], scalar1=1.0, scalar2=0.0,
                            op0=ALU.mult, op1=ALU.add, accum_out=sacc,
                        )
                        nc.vector.scalar_tensor_tensor(
                            out=sq[:, :cc], in0=Hs, scalar=1.0, in1=Hs,
                            op0=ALU.mult, op1=ALU.mult, accum_out=qacc,
                        )

        # ---------------- BN2 statistics (the only collective) ----------------
        for m in range(8):
            eng = nc.vector
            eng.reduce_sum(
                stats2_sb[:, m : m + 1],
                sum_parts[:, m * NCH : (m + 1) * NCH],
                axis=X,
            )
            eng.reduce_sum(
                stats2_sb[:, 8 + m : 9 + m],
                sumsq_parts[:, m * NCH : (m + 1) * NCH],
                axis=X,
            )
        nc.sync.dma_start(out=cc2_in[:], in_=stats2_sb[:])
        nc.gpsimd.collective_compute(
            "AllReduce",
            mybir.AluOpType.add,
            replica_groups=RG,
            ins=[cc2_in[:].opt()],
            outs=[cc2_out[:].opt()],
        )
        nc.sync.dma_start(out=stats2g[:], in_=cc2_out[:])

        mean2 = work.tile([128, 8], fp32, tag="mean2", name="mean2")
        nc.vector.tensor_scalar_mul(mean2[:], stats2g[:, 0:8], 1.0 / N_TOTAL)
        var2 = work.tile([128, 8], fp32, tag="var2", name="var2")
        nc.vector.tensor_scalar_mul(var2[:], stats2g[:, 8:16], 1.0 / N_TOTAL)
        msq2 = work.tile([128, 8], fp32, tag="msq2", name="msq2")
        nc.vector.tensor_mul(msq2[:], mean2[:], mean2[:])
        nc.vector.tensor_sub(var2[:], var2[:], msq2[:])
        nc.vector.tensor_scalar_add(var2[:], var2[:], BN_EPS)
        sd2 = work.tile([128, 8], fp32, tag="sd2", name="sd2")
        nc.scalar.sqrt(sd2[:], var2[:])
        rstd2 = work.tile([128, 8], fp32, tag="rstd2", name="rstd2")
        nc.vector.reciprocal(rstd2[:], sd2[:])
        nc.vector.tensor_mul(bn2_scale[:], rstd2[:], vg[:, 16:24])
        t22 = work.tile([128, 8], fp32, tag="t22", name="t22")
        nc.vector.tensor_mul(t22[:], mean2[:], bn2_scale[:])
        nc.vector.tensor_sub(bn2_bias[:], vg[:, 24:32], t22[:])

        # -------- pass C: BN2+ReLU -> L3 -> L4 -> log_softmax --------
        with tc.tile_pool(name="ph3", bufs=2, space="PSUM") as ph3, \
             tc.tile_pool(name="plog", bufs=2, space="PSUM") as plog, \
             tc.tile_pool(name="ptr2", bufs=3, space="PSUM") as ptr2:
            for j, (c0, cc) in enumerate(CH_LIST):
                a2 = [
                    acts.tile([128, CH], bf16, tag=f"a1_{k}", name=f"a2_{j}_{k}")
                    for k in range(8)
                ]
                for k in range(8):
                    src = Hbig[:, k * R + c0 : k * R + c0 + cc]
                    if k < 4:
                        nc.scalar.activation(
                            a2[k][:, :cc], src, AF.Relu,
                            bias=bn2_bias[:, k : k + 1],
                            scale=bn2_scale[:, k : k + 1],
                        )
                    else:
                        tmp = sqs.tile(
                            [128, CH], bf16, tag=f"sq{k % 4}", name=f"af{j}_{k}"
                        )
                        nc.vector.tensor_scalar(
                            out=tmp[:, :cc], in0=src,
                            scalar1=bn2_scale[:, k : k + 1],
                            scalar2=bn2_bias[:, k : k + 1],
                            op0=mybir.AluOpType.mult, op1=mybir.AluOpType.add,
                        )
                        nc.vector.tensor_scalar_max(a2[k][:, :cc], tmp[:, :cc], 0.0)
                h3 = [
                    h3pool.tile([128, CH], bf16, tag=f"h3_{m3}", name=f"h3_{j}_{m3}")
                    for m3 in range(2)
                ]
                for m3 in range(2):
                    h3_ps = ph3.tile([128, CH], fp32, tag="ph3", name=f"h3ps{j}_{m3}")
                    for k in range(8):
                        nc.tensor.matmul(
                            h3_ps[:, :cc],
                            lhsT=wl1T[:, k * HM + m3 * 128 : k * HM + (m3 + 1) * 128],
                            rhs=a2[k][:, :cc],
                            start=(k == 0),
                            stop=(k == 7),
                        )
                    nc.scalar.activation(
                        h3[m3][:, :cc], h3_ps[:, :cc], AF.Relu,
                        bias=vg[:, 32 + m3 : 33 + m3], scale=1.0,
                    )
                lg_ps = plog.tile([C, CH], fp32, tag="plog", name=f"lg{j}")
                nc.tensor.matmul(
                    lg_ps[:, :cc],
                    lhsT=bl2[:],
                    rhs=ones_row[:, :cc],
                    start=True,
                    stop=False,
                )
                for k3 in range(2):
                    nc.tensor.matmul(
                        lg_ps[:, :cc],
                        lhsT=wl2T[:, k3 * C : (k3 + 1) * C],
                        rhs=h3[k3][:, :cc],
                        start=False,
                        stop=(k3 == 1),
                    )
                lg_sb = lgpool.tile([C, CH], fp32, tag="lg", name=f"lgs{j}")
                nc.vector.tensor_copy(lg_sb[:, :cc], lg_ps[:, :cc])
                # transpose logits to row-major and collect into rows_all
                nt = (cc + 127) // 128
                for t in range(nt):
                    rt0 = t * 128
                    rt = min(128, cc - rt0)
                    tg = (c0 + rt0) // 128
                    tp_ps = ptr2.tile([128, C], fp32, tag="ptr2", name=f"sm{j}_{t}")
                    nc.tensor.transpose(
                        tp_ps[:rt, :],
                        lg_sb[:, rt0 : rt0 + rt],
                        identity[:C, :C],
                    )
                    nc.vector.tensor_copy(
                        rows_all[:rt, tg * C : (tg + 1) * C], tp_ps[:rt, :]
                    )

            # ---- batched log_softmax over all row tiles ----
            # logits are O(10), so exp() without max-subtraction is safe in f32
            nc.scalar.activation(e_all[:], rows_all[:], AF.Exp)
            nc.vector.reduce_sum(
                sums_all[:],
                e_all[:].rearrange("p (t c) -> p t c", c=C),
                axis=X,
            )
            nc.scalar.activation(lse_all[:], sums_all[:], AF.Ln)
            nc.vector.tensor_sub(
                res_all[:].rearrange("p (t c) -> p t c", c=C),
                rows_all[:].rearrange("p (t c) -> p t c", c=C),
                lse_all[:].to_broadcast([128, NRTT, C]),
            )
            nc.sync.dma_start(
                out=out_d[: NFULL * 128].rearrange("(t p) c -> p t c", p=128),
                in_=res_all[:, : NFULL * C],
            )
            rtail = R - NFULL * 128
            if rtail:
                nc.sync.dma_start(
                    out=out_d[NFULL * 128 :],
                    in_=res_all[:rtail, NFULL * C :],
                )


def _get_nc():
    if "nc" not in _CACHE:
        _CACHE["nc"] = _build(os.environ.get("KERNEL_STAGE", "full"))
    return _CACHE["nc"]


def make_in_maps(inputs):
    import ml_dtypes

    BF = ml_dtypes.bfloat16
    F8 = ml_dtypes.float8_e4m3
    f32 = np.float32

    x = np.ascontiguousarray(np.asarray(inputs["x"]), dtype=f32)
    W1 = np.asarray(inputs["W1"], dtype=f32)
    W2 = np.asarray(inputs["W2"], dtype=f32)
    Wl1 = np.asarray(inputs["Wl1"], dtype=f32)
    Wl2 = np.asarray(inputs["Wl2"], dtype=f32)

    # full-x fp8 tiles [p, t, d]: x[t*128 + p, d], plus a ones column
    xpad = np.zeros((NT * 128, DIN), f32)
    xpad[:N_TOTAL] = x
    xf8 = np.empty((128, NT, TD), F8)
    xf8[:, :, :DIN] = xpad.reshape(NT, 128, DIN).transpose(1, 0, 2).astype(F8)
    xf8[:, :, DIN] = f32(1.0)

    w1T = np.ascontiguousarray(W1.T).astype(BF)
    w2T = np.ascontiguousarray(W2.T).astype(BF)
    wl1T = np.ascontiguousarray(Wl1.T).astype(BF)
    wl2T = np.ascontiguousarray(Wl2.T).astype(BF)
    bl2 = np.asarray(inputs["bl2"], f32).reshape(1, C).astype(BF)

    vg = np.zeros((128, 34), f32)
    vg[:, 0:8] = np.asarray(inputs["g1"], f32).reshape(8, 128).T
    vg[:, 8:16] = np.asarray(inputs["be1"], f32).reshape(8, 128).T
    vg[:, 16:24] = np.asarray(inputs["g2"], f32).reshape(8, 128).T
    vg[:, 24:32] = np.asarray(inputs["be2"], f32).reshape(8, 128).T
    vg[:, 32:34] = np.asarray(inputs["bl1"], f32).reshape(2, 128).T

    return [
        {
            "xT": np.ascontiguousarray(x[i * R : (i + 1) * R].T).astype(BF),
            "xf8": xf8,
            "w1T": w1T,
            "w2T": w2T,
            "wl1T": wl1T,
            "wl2T": wl2T,
            "bl2": bl2,
            "vg": vg,
        }
        for i in range(NCORES)
    ]


def kernel(**inputs):
    from concourse.bass_utils import run_bass_kernel_spmd

    nc = _get_nc()
    in_maps = make_in_maps(inputs)
    res = run_bass_kernel_spmd(nc, in_maps, core_ids=list(range(NCORES)))
    return np.concatenate([r["out"] for r in res.results], axis=0).astype(np.float32)


# revision 16
# speedup vs baseline: 1.3905x; 1.0783x over previous
"""Trainium2 Bass kernel for ChebyNet (K=1) forward pass.

ChebConv with K=1 reduces to a plain linear layer on the T0 (identity) term,
so edge_index / edge_weight never enter the math. The network is:

    h1 = x @ W1.T (+b1, dropped: BN cancels additive bias) -> BN -> ReLU
    h2 = h1 @ W2.T (+b2, dropped)                          -> BN -> ReLU
    h3 = relu(h2 @ Wl1.T + bl1)
    out = log_softmax(h3 @ Wl2.T + bl2, axis=1)

Sharding: nodes (N=50000) split across 8 NeuronCores (R=6250 rows each).

Key design points vs a naive port:
  - All GEMMs run in bf16 (full PE rate; fp32r is half rate on TRN2).
    Weights are pre-transposed and pre-cast to bf16 on the host.
  - BN1 stats are computed locally on EVERY core from a replicated fp8
    copy of the full x via the Gram matrix (mean/var of x@W1.T are a
    bilinear form of Gram(x)). This removes the first AllReduce entirely,
    so the cross-core rendezvous barrier (~100us of start skew) overlaps
    useful work instead of stalling the main pass.
  - h2 stays resident in SBUF in bf16 ([128, 8*6250] = 100KB/partition),
    eliminating the 51MB DRAM spill+reload between the BN2-stats pass and
    the normalize pass. Only BN2 stats need a (tiny, 8KB) AllReduce.
  - Elementwise work (BN+ReLU, evictions, sum-of-squares) is spread
    across scalar/vector/gpsimd so the PE stays the bottleneck.
"""

import os
import sys

sys.path.insert(0, "/opt/trn_rl_repo")

import numpy as np

NCORES = 8
N_TOTAL = 50000
R = N_TOTAL // NCORES  # 6250 rows per core
DIN = 128
H = 1024
HM = 256
C = 10
BN_EPS = 1e-5
CH = 512  # row-chunk (matmul moving dim)

NT = (N_TOTAL + 127) // 128  # 391 full-x row tiles -> pad to 392
NT += NT % 2
TD = DIN + 1  # 129: x tile plus a ones column (Gram + colsum in one matmul)

NRTT = (R + 127) // 128  # 49 row tiles per core
NFULL = R // 128  # 48 full tiles
CH_LIST = [(i * CH, min(CH, R - i * CH)) for i in range((R + CH - 1) // CH)]
NCH = len(CH_LIST)
if os.environ.get("CH_LIMIT"):
    CH_LIST = CH_LIST[: int(os.environ["CH_LIMIT"])]

_CACHE = {}


def _build(stage="full"):
    import concourse.bass as bass  # noqa: F401
    import concourse.tile as tile
    import concourse.mybir as mybir
    from concourse import bacc
    from concourse.masks import make_identity

    fp32 = mybir.dt.float32
    bf16 = mybir.dt.bfloat16
    fp8 = mybir.dt.float8e4
    AF = mybir.ActivationFunctionType
    ALU = mybir.AluOpType
    X = mybir.AxisListType.X

    nc = bacc.Bacc(num_devices=NCORES, debug=False)

    xT_d = nc.dram_tensor("xT", [DIN, R], bf16, kind="ExternalInput")
    xf8_d = nc.dram_tensor("xf8", [128, NT, TD], fp8, kind="ExternalInput")
    w1T_d = nc.dram_tensor("w1T", [DIN, H], bf16, kind="ExternalInput")
    w2T_d = nc.dram_tensor("w2T", [H, H], bf16, kind="ExternalInput")
    wl1T_d = nc.dram_tensor("wl1T", [H, HM], bf16, kind="ExternalInput")
    wl2T_d = nc.dram_tensor("wl2T", [HM, C], bf16, kind="ExternalInput")
    bl2_d = nc.dram_tensor("bl2", [1, C], bf16, kind="ExternalInput")
    # vg: [128, 34] f32: cols 0-7 g1, 8-15 be1, 16-23 g2, 24-31 be2, 32-33 bl1
    # (per-feature vectors laid out as [p, m] with feature = m*128 + p)
    vg_d = nc.dram_tensor("vg", [128, 34], fp32, kind="ExternalInput")
    out_d = nc.dram_tensor("out", [R, C], fp32, kind="ExternalOutput")

    RG = [list(range(NCORES))]

    with tile.TileContext(nc) as tc:
        with (
            tc.tile_pool(name="persist", bufs=1) as persist,
            tc.tile_pool(name="work", bufs=2) as work,
            tc.tile_pool(name="dram", bufs=1, space="DRAM") as dram,
        ):
            # ---------------- persistent tiles -----------------
            identity = persist.tile([128, 128], fp32, tag="identity", name="identity")
            make_identity(nc, identity[:])
            ones_col = persist.tile([128, 1], fp32, tag="ones", name="ones_col")
            nc.vector.memset(ones_col[:], 1.0)
            ones_row = persist.tile([1, CH], bf16, tag="onesr", name="ones_row")
            nc.vector.memset(ones_row[:], 1.0)

            xT = persist.tile([128, R], bf16, tag="xT", name="xT")
            w1T = persist.tile([128, H], bf16, tag="w1T", name="w1T")
            w2T = persist.tile([128, 8 * H], bf16, tag="w2T", name="w2T")
            wl1T = persist.tile([128, 8 * HM], bf16, tag="wl1T", name="wl1T")
            wl2T = persist.tile([128, 2 * C], bf16, tag="wl2T", name="wl2T")
            bl2 = persist.tile([1, C], bf16, tag="bl2", name="bl2")
            vg = persist.tile([128, 34], fp32, tag="vg", name="vg")

            bn1_scale = persist.tile([128, 8], fp32, tag="bn1s", name="bn1_scale")
            bn1_bias = persist.tile([128, 8], fp32, tag="bn1b", name="bn1_bias")
            bn2_scale = persist.tile([128, 8], fp32, tag="bn2s", name="bn2_scale")
            bn2_bias = persist.tile([128, 8], fp32, tag="bn2b", name="bn2_bias")

            sum_parts = persist.tile([128, 8 * NCH], fp32, tag="sump", name="sum_parts")
            sumsq_parts = persist.tile(
                [128, 8 * NCH], fp32, tag="sumq", name="sumsq_parts"
            )
            stats2_sb = persist.tile([128, 16], fp32, tag="st2", name="stats2_sb")
            stats2g = persist.tile([128, 16], fp32, tag="st2g", name="stats2g")

            rows_all = persist.tile([128, NRTT * C], fp32, tag="rows_all", name="rows_all")
            nc.vector.memset(rows_all[:], 0.0)
            e_all = persist.tile([128, NRTT * C], fp32, tag="e_all", name="e_all")
            res_all = persist.tile([128, NRTT * C], fp32, tag="res_all", name="res_all")
            sums_all = persist.tile([128, NRTT], fp32, tag="sums_all", name="sums_all")
            lse_all = persist.tile([128, NRTT], fp32, tag="lse_all", name="lse_all")

            cc0_in = dram.tile([128, 16], fp32, name="cc0_in")
            cc0_out = dram.tile([128, 16], fp32, name="cc0_out")
            cc2_in = dram.tile([128, 16], fp32, name="cc2_in")
            cc2_out = dram.tile([128, 16], fp32, name="cc2_out")


            # ============ startup: loads + replicated Gram -> BN1 stats ============
            with tc.tile_pool(name="boot", bufs=1) as boot, \
                 tc.tile_pool(name="pg", bufs=1, space="PSUM") as pg, \
                 tc.tile_pool(name="pv", bufs=2, space="PSUM") as pv, \
                 tc.tile_pool(name="psmall", bufs=4, space="PSUM") as psmall:
                xf8 = boot.tile([128, NT, TD], fp8, tag="xf8", name="xf8")
                # xf8 gates BN1 stats -> issue its pieces first, one per engine
                dma_eng = [nc.sync, nc.scalar, nc.gpsimd, nc.sync]
                NP = 4
                PT = NT // NP  # 98 tiles per piece
                for P in range(NP):
                    dma_eng[P].dma_start(
                        out=xf8[:, P * PT : (P + 1) * PT, :],
                        in_=xf8_d[:, P * PT : (P + 1) * PT, :],
                    )
                # remaining loads ride behind the xf8 pieces
                nc.sync.dma_start(out=xT[:], in_=xT_d[:])
                nc.scalar.dma_start(out=w1T[:], in_=w1T_d[:])
                nc.gpsimd.dma_start(
                    out=w2T[:], in_=w2T_d[:].rearrange("(k p) h -> p k h", p=128)
                )
                nc.scalar.dma_start(
                    out=wl1T[:], in_=wl1T_d[:].rearrange("(k p) h -> p k h", p=128)
                )
                nc.sync.dma_start(
                    out=wl2T[:], in_=wl2T_d[:].rearrange("(k p) c -> p k c", p=128)
                )
                nc.scalar.dma_start(out=bl2[:], in_=bl2_d[:])
                nc.gpsimd.dma_start(out=vg[:], in_=vg_d[:])

                # skew-eater: a dummy AllReduce launched right after the load
                # DMAs are enqueued (collective_compute blocks the gpsimd
                # queue, so it must come after gpsimd's dma_starts). Its
                # result is consumed mid-pass-B, so cross-core start skew is
                # absorbed while compute runs instead of stalling AR2.
                nc.vector.memset(stats2_sb[:], 0.0)
                nc.sync.dma_start(out=cc0_in[:], in_=stats2_sb[:])
                nc.gpsimd.collective_compute(
                    "AllReduce",
                    ALU.add,
                    replica_groups=RG,
                    ins=[cc0_in[:].opt()],
                    outs=[cc0_out[:].opt()],
                )

                # Gram of the full x: G[d1,d2] = sum_n x[n,d1] x[n,d2]; the
                # ones column makes out[:, 128] the column sums. Grouped per
                # DMA piece so matmuls start as soon as each piece lands.
                gram_ps = pg.tile([128, TD], fp32, tag="gram", name="gram_ps")
                for t in range(NT):
                    nc.tensor.matmul(
                        gram_ps[:],
                        lhsT=xf8[:, t, 0:DIN],
                        rhs=xf8[:, t, :],
                        start=(t == 0),
                        stop=(t == NT - 1),
                    )

                # BN1 stats from Gram:  mean = W1 @ (cs/N),
                # E[h1^2]_j = (w_j^T G w_j)/N = colsum(w1T .* (G @ W1^T))_j / N
                g_bf = boot.tile([128, DIN], bf16, tag="g_bf", name="g_bf")
                nc.scalar.copy(g_bf[:], gram_ps[:, 0:DIN])
                mean_f = work.tile([128, 1], fp32, tag="meanf", name="mean_f")
                nc.scalar.mul(mean_f[:], gram_ps[:, DIN : DIN + 1], 1.0 / N_TOTAL)
                mean_bf = boot.tile([128, 1], bf16, tag="meanbf", name="mean_bf")
                nc.scalar.copy(mean_bf[:], mean_f[:])

                V_sb = boot.tile([128, H], fp32, tag="Vsb", name="V_sb")
                for half in range(2):
                    mp = pv.tile([128, 512], fp32, tag="pv", name=f"mp{half}")
                    nc.tensor.matmul(
                        mp[:],
                        lhsT=g_bf[:],
                        rhs=w1T[:, half * 512 : (half + 1) * 512],
                        start=True,
                        stop=True,
                    )
                    nc.vector.tensor_mul(
                        V_sb[:, half * 512 : (half + 1) * 512],
                        w1T[:, half * 512 : (half + 1) * 512],
                        mp[:],
                    )
                E2 = work.tile([128, 8], fp32, tag="E2", name="E2")
                WXM = work.tile([128, 8], fp32, tag="WXM", name="WXM")
                for m in range(8):
                    sl = slice(m * 128, (m + 1) * 128)
                    e2_ps = psmall.tile([128, 1], fp32, tag="psm", name=f"e2{m}")
                    nc.tensor.matmul(
                        e2_ps[:], lhsT=V_sb[:, sl], rhs=ones_col[:],
                        start=True, stop=True,
                    )
                    nc.vector.tensor_copy(E2[:, m : m + 1], e2_ps[:])
                    wxm_ps = psmall.tile([128, 1], fp32, tag="psm", name=f"wxm{m}")
                    nc.tensor.matmul(
                        wxm_ps[:], lhsT=w1T[:, sl], rhs=mean_bf[:],
                        start=True, stop=True,
                    )
                    nc.vector.tensor_copy(WXM[:, m : m + 1], wxm_ps[:])

                # vectorized coeffs: scale = g1 / sqrt(E2/N - WXM^2 + eps)
                #                    bias  = be1 - WXM * scale
                var_t = work.tile([128, 8], fp32, tag="var1", name="var1")
                nc.vector.tensor_scalar_mul(var_t[:], E2[:], 1.0 / N_TOTAL)
                msq = work.tile([128, 8], fp32, tag="msq1", name="msq1")
                nc.vector.tensor_mul(msq[:], WXM[:], WXM[:])
                nc.vector.tensor_sub(var_t[:], var_t[:], msq[:])
                nc.vector.tensor_scalar_add(var_t[:], var_t[:], BN_EPS)
                sd = work.tile([128, 8], fp32, tag="sd1", name="sd1")
                nc.scalar.sqrt(sd[:], var_t[:])
                rstd = work.tile([128, 8], fp32, tag="rstd1", name="rstd1")
                nc.vector.reciprocal(rstd[:], sd[:])
                nc.vector.tensor_mul(bn1_scale[:], rstd[:], vg[:, 0:8])
                t2 = work.tile([128, 8], fp32, tag="t21", name="t21")
                nc.vector.tensor_mul(t2[:], WXM[:], bn1_scale[:])
                nc.vector.tensor_sub(bn1_bias[:], vg[:, 8:16], t2[:])

            if stage == "s1":
                dummy = work.tile([128, C], fp32, tag="dummy", name="dummy")
                nc.vector.tensor_copy(dummy[:, 0:8], bn1_scale[:])
                nc.vector.tensor_copy(dummy[:, 8:10], bn1_bias[:, 0:2])
                for r0 in range(0, R, 128):
                    rr = min(128, R - r0)
                    nc.sync.dma_start(out=out_d[r0 : r0 + rr, :], in_=dummy[:rr, :])
            else:
                _build_rest(
                    nc, tc, stage, mybir, fp32, bf16, AF, ALU, X,
                    persist, work, dram, identity, ones_row, bl2, vg,
                    w1T, w2T, wl1T, wl2T, xT,
                    bn1_scale, bn1_bias, bn2_scale, bn2_bias,
                    sum_parts, sumsq_parts, stats2_sb, stats2g,
                    rows_all, e_all, res_all, sums_all, lse_all,
                    cc0_out, cc2_in, cc2_out, out_d, RG,
                )

    nc.finalize()
    return nc


def _build_rest(
    nc, tc, stage, mybir, fp32, bf16, AF, ALU, X,
    persist, work, dram, identity, ones_row, bl2, vg,
    w1T, w2T, wl1T, wl2T, xT,
    bn1_scale, bn1_bias, bn2_scale, bn2_bias,
    sum_parts, sumsq_parts, stats2_sb, stats2g,
    rows_all, e_all, res_all, sums_all, lse_all,
    cc0_out, cc2_in, cc2_out, out_d, RG,
):
    with (
        tc.tile_pool(name="hpool", bufs=1) as hpool,
        tc.tile_pool(name="acts", bufs=1) as acts,
        tc.tile_pool(name="sqs", bufs=2) as sqs,
        tc.tile_pool(name="h3pool", bufs=2) as h3pool,
        tc.tile_pool(name="lgpool", bufs=2) as lgpool,
    ):
        Hbig = hpool.tile([128, 8 * R], bf16, tag="Hbig", name="Hbig")

        # ------------- pass B: L1 -> BN1+ReLU -> L2, h2 -> SBUF -------------
        skew_sb = work.tile([128, 16], fp32, tag="skew", name="skew_sb")
        skew_c = work.tile([128, 1], fp32, tag="skewc", name="skew_c")
        with tc.tile_pool(name="ph1", bufs=3, space="PSUM") as ph1, \
             tc.tile_pool(name="ph2", bufs=3, space="PSUM") as ph2:
            for j, (c0, cc) in enumerate(CH_LIST):
                if j == 8:
                    nc.sync.dma_start(out=skew_sb[:], in_=cc0_out[:])
                    nc.vector.tensor_copy(skew_c[:], skew_sb[:, 0:1])
                a1 = [
                    acts.tile([128, CH], bf16, tag=f"a1_{k}", name=f"a1_{j}_{k}")
                    for k in range(8)
                ]
                for m in range(8):
                    sl = slice(m * 128, (m + 1) * 128)
                    h1_ps = ph1.tile([128, CH], fp32, tag="ph1", name=f"h1ps{j}_{m}")
                    nc.tensor.matmul(
                        h1_ps[:, :cc],
                        lhsT=w1T[:, sl],
                        rhs=xT[:, c0 : c0 + cc],
                        start=True,
                        stop=True,
                    )
                    nc.scalar.activation(
                        a1[m][:, :cc],
                        h1_ps[:, :cc],
                        AF.Relu,
                        bias=bn1_bias[:, m : m + 1],
                        scale=bn1_scale[:, m : m + 1],
                    )
                for m in range(8):
                    h2_ps = ph2.tile([128, CH], fp32, tag="ph2", name=f"h2ps{j}_{m}")
                    for k in range(8):
                        nc.tensor.matmul(
                            h2_ps[:, :cc],
                            lhsT=w2T[:, k * H + m * 128 : k * H + (m + 1) * 128],
                            rhs=a1[k][:, :cc],
                            start=(k == 0),
                            stop=(k == 7),
                        )
                    Hs = Hbig[:, m * R + c0 : m * R + c0 + cc]
                    sacc = sum_parts[:, m * NCH + j : m * NCH + j + 1]
                    qacc = sumsq_parts[:, m * NCH + j : m * NCH + j + 1]
                    sq = sqs.tile([128, CH], bf16, tag=f"sq{m % 4}", name=f"sq{j}_{m}")
                    if m < 4:
                        # evict+sum and sum-of-squares both on scalar
                        nc.scalar.activation(
                            Hs, h2_ps[:, :cc], AF.Identity,
                            bias=0.0, scale=1.0, accum_out=sacc,
                        )
                        nc.scalar.activation(
                            sq[:, :cc], Hs, AF.Square,
                            bias=0.0, scale=1.0, accum_out=qacc,
                        )
                    else:
                        # evict+sum and sum-of-squares both on vector
                        nc.vector.tensor_scalar(
                            out=Hs, in0=h2_ps[:, :cc], scalar1=1.0, scalar2=0.0,
                            op0=ALU.mult, op1=ALU.add, accum_out=sacc,
                        )
                        nc.vector.scalar_tensor_tensor(
                            out=sq[:, :cc], in0=Hs, scalar=1.0, in1=Hs,
                            op0=ALU.mult, op1=ALU.mult, accum_out=qacc,
                        )

        # ---------------- BN2 statistics (the only collective) ----------------
        for m in range(8):
            eng = nc.vector
            eng.reduce_sum(
                stats2_sb[:, m : m + 1],
                sum_parts[:, m * NCH : (m + 1) * NCH],
                axis=X,
            )
            eng.reduce_sum(
                stats2_sb[:, 8 + m : 9 + m],
                sumsq_parts[:, m * NCH : (m + 1) * NCH],
                axis=X,
            )
        nc.sync.dma_start(out=cc2_in[:], in_=stats2_sb[:])
        nc.gpsimd.collective_compute(
            "AllReduce",
            mybir.AluOpType.add,
            replica_groups=RG,
            ins=[cc2_in[:].opt()],
            outs=[cc2_out[:].opt()],
        )
        nc.sync.dma_start(out=stats2g[:], in_=cc2_out[:])

        mean2 = work.tile([128, 8], fp32, tag="mean2", name="mean2")
        nc.vector.tensor_scalar_mul(mean2[:], stats2g[:, 0:8], 1.0 / N_TOTAL)
        var2 = work.tile([128, 8], fp32, tag="var2", name="var2")
        nc.vector.tensor_scalar_mul(var2[:], stats2g[:, 8:16], 1.0 / N_TOTAL)
        msq2 = work.tile([128, 8], fp32, tag="msq2", name="msq2")
        nc.vector.tensor_mul(msq2[:], mean2[:], mean2[:])
        nc.vector.tensor_sub(var2[:], var2[:], msq2[:])
        nc.vector.tensor_scalar_add(var2[:], var2[:], BN_EPS)
        sd2 = work.tile([128, 8], fp32, tag="sd2", name="sd2")
        nc.scalar.sqrt(sd2[:], var2[:])
        rstd2 = work.tile([128, 8], fp32, tag="rstd2", name="rstd2")
        nc.vector.reciprocal(rstd2[:], sd2[:])
        nc.vector.tensor_mul(bn2_scale[:], rstd2[:], vg[:, 16:24])
        t22 = work.tile([128, 8], fp32, tag="t22", name="t22")
        nc.vector.tensor_mul(t22[:], mean2[:], bn2_scale[:])
        nc.vector.tensor_sub(bn2_bias[:], vg[:, 24:32], t22[:])

        # -------- pass C: BN2+ReLU -> L3 -> L4 -> log_softmax --------
        with tc.tile_pool(name="ph3", bufs=2, space="PSUM") as ph3, \
             tc.tile_pool(name="plog", bufs=2, space="PSUM") as plog, \
             tc.tile_pool(name="ptr2", bufs=3, space="PSUM") as ptr2:
            for j, (c0, cc) in enumerate(CH_LIST):
                a2 = [
                    acts.tile([128, CH], bf16, tag=f"a1_{k}", name=f"a2_{j}_{k}")
                    for k in range(8)
                ]
                for k in range(8):
                    src = Hbig[:, k * R + c0 : k * R + c0 + cc]
                    if k < 4:
                        nc.scalar.activation(
                            a2[k][:, :cc], src, AF.Relu,
                            bias=bn2_bias[:, k : k + 1],
                            scale=bn2_scale[:, k : k + 1],
                        )
                    else:
                        tmp = sqs.tile(
                            [128, CH], bf16, tag=f"sq{k % 4}", name=f"af{j}_{k}"
                        )
                        nc.vector.tensor_scalar(
                            out=tmp[:, :cc], in0=src,
                            scalar1=bn2_scale[:, k : k + 1],
                            scalar2=bn2_bias[:, k : k + 1],
                            op0=mybir.AluOpType.mult, op1=mybir.AluOpType.add,
                        )
                        nc.vector.tensor_scalar_max(a2[k][:, :cc], tmp[:, :cc], 0.0)
                h3 = [
                    h3pool.tile([128, CH], bf16, tag=f"h3_{m3}", name=f"h3_{j}_{m3}")
                    for m3 in range(2)
                ]
                for m3 in range(2):
                    h3_ps = ph3.tile([128, CH], fp32, tag="ph3", name=f"h3ps{j}_{m3}")
                    for k in range(8):
                        nc.tensor.matmul(
                            h3_ps[:, :cc],
                            lhsT=wl1T[:, k * HM + m3 * 128 : k * HM + (m3 + 1) * 128],
                            rhs=a2[k][:, :cc],
                            start=(k == 0),
                            stop=(k == 7),
                        )
                    nc.scalar.activation(
                        h3[m3][:, :cc], h3_ps[:, :cc], AF.Relu,
                        bias=vg[:, 32 + m3 : 33 + m3], scale=1.0,
                    )
                lg_ps = plog.tile([C, CH], fp32, tag="plog", name=f"lg{j}")
                nc.tensor.matmul(
                    lg_ps[:, :cc],
                    lhsT=bl2[:],
                    rhs=ones_row[:, :cc],
                    start=True,
                    stop=False,
                )
                for k3 in range(2):
                    nc.tensor.matmul(
                        lg_ps[:, :cc],
                        lhsT=wl2T[:, k3 * C : (k3 + 1) * C],
                        rhs=h3[k3][:, :cc],
                        start=False,
                        stop=(k3 == 1),
                    )
                lg_sb = lgpool.tile([C, CH], fp32, tag="lg", name=f"lgs{j}")
                nc.vector.tensor_copy(lg_sb[:, :cc], lg_ps[:, :cc])
                # transpose logits to row-major and collect into rows_all
                nt = (cc + 127) // 128
                for t in range(nt):
                    rt0 = t * 128
                    rt = min(128, cc - rt0)
                    tg = (c0 + rt0) // 128
                    tp_ps = ptr2.tile([128, C], fp32, tag="ptr2", name=f"sm{j}_{t}")
                    nc.tensor.transpose(
                        tp_ps[:rt, :],
                        lg_sb[:, rt0 : rt0 + rt],
                        identity[:C, :C],
                    )
                    nc.vector.tensor_copy(
                        rows_all[:rt, tg * C : (tg + 1) * C], tp_ps[:rt, :]
                    )

            # ---- batched log_softmax over all row tiles ----
            # logits are O(10), so exp() without max-subtraction is safe in f32
            nc.scalar.activation(e_all[:], rows_all[:], AF.Exp)
            nc.vector.reduce_sum(
                sums_all[:],
                e_all[:].rearrange("p (t c) -> p t c", c=C),
                axis=X,
            )
            nc.scalar.activation(lse_all[:], sums_all[:], AF.Ln)
            nc.vector.tensor_sub(
                res_all[:].rearrange("p (t c) -> p t c", c=C),
                rows_all[:].rearrange("p (t c) -> p t c", c=C),
                lse_all[:].to_broadcast([128, NRTT, C]),
            )
            nc.sync.dma_start(
                out=out_d[: NFULL * 128].rearrange("(t p) c -> p t c", p=128),
                in_=res_all[:, : NFULL * C],
            )
            rtail = R - NFULL * 128
            if rtail:
                nc.sync.dma_start(
                    out=out_d[NFULL * 128 :],
                    in_=res_all[:rtail, NFULL * C :],
                )


def _get_nc():
    if "nc" not in _CACHE:
        _CACHE["nc"] = _build(os.environ.get("KERNEL_STAGE", "full"))
    return _CACHE["nc"]


def make_in_maps(inputs):
    import ml_dtypes

    BF = ml_dtypes.bfloat16
    F8 = ml_dtypes.float8_e4m3
    f32 = np.float32

    x = np.ascontiguousarray(np.asarray(inputs["x"]), dtype=f32)
    W1 = np.asarray(inputs["W1"], dtype=f32)
    W2 = np.asarray(inputs["W2"], dtype=f32)
    Wl1 = np.asarray(inputs["Wl1"], dtype=f32)
    Wl2 = np.asarray(inputs["Wl2"], dtype=f32)

    # full-x fp8 tiles [p, t, d]: x[t*128 + p, d], plus a ones column
    xpad = np.zeros((NT * 128, DIN), f32)
    xpad[:N_TOTAL] = x
    xf8 = np.empty((128, NT, TD), F8)
    xf8[:, :, :DIN] = xpad.reshape(NT, 128, DIN).transpose(1, 0, 2).astype(F8)
    xf8[:, :, DIN] = f32(1.0)

    w1T = np.ascontiguousarray(W1.T).astype(BF)
    w2T = np.ascontiguousarray(W2.T).astype(BF)
    wl1T = np.ascontiguousarray(Wl1.T).astype(BF)
    wl2T = np.ascontiguousarray(Wl2.T).astype(BF)
    bl2 = np.asarray(inputs["bl2"], f32).reshape(1, C).astype(BF)

    vg = np.zeros((128, 34), f32)
    vg[:, 0:8] = np.asarray(inputs["g1"], f32).reshape(8, 128).T
    vg[:, 8:16] = np.asarray(inputs["be1"], f32).reshape(8, 128).T
    vg[:, 16:24] = np.asarray(inputs["g2"], f32).reshape(8, 128).T
    vg[:, 24:32] = np.asarray(inputs["be2"], f32).reshape(8, 128).T
    vg[:, 32:34] = np.asarray(inputs["bl1"], f32).reshape(2, 128).T

    return [
        {
            "xT": np.ascontiguousarray(x[i * R : (i + 1) * R].T).astype(BF),
            "xf8": xf8,
            "w1T": w1T,
            "w2T": w2T,
            "wl1T": wl1T,
            "wl2T": wl2T,
            "bl2": bl2,
            "vg": vg,
        }
        for i in range(NCORES)
    ]


def kernel(**inputs):
    from concourse.bass_utils import run_bass_kernel_spmd

    nc = _get_nc()
    in_maps = make_in_maps(inputs)
    res = run_bass_kernel_spmd(nc, in_maps, core_ids=list(range(NCORES)))
    return np.concatenate([r["out"] for r in res.results], axis=0).astype(np.float32)
